# revision 3
# baseline (speedup 1.0000x reference)
"""v3 Trainium2 Bass kernel.

Scheme (per core c of 8, per topo batch b of 8):
  gather v (remote-DMA all-gather, NOT gpsimd collective) -> layernorm stats
  (Newton-1 rsqrt) -> topo self-attention on the span -> per-neuron
  self-attention for this core's TL=16 neurons (bf16 matmuls; k*rs scale
  built as a PE matmul krep = krsT @ dsel and applied as one DVE multiply
  per 512-col bank, then one ACT exp per bank) -> masked affine (fp32)
  -> adaptive gelu on the sel-placed [128,1] column -> remote_dma_broadcast
  to all 8 cores' SBUF (slot k -> tpb my^k), wait on per-batch remote sem.

v3 changes vs v2 (327.7us):
  - All heavy matmuls bf16 (1 cy/row vs 4 for fp32); affine stays fp32.
  - gpsimd AllGather (15us/call in the cost model) replaced by 8
    single-dest remote_dma_broadcast preps + one trigger per batch
    (~1-2us); per-batch remote semaphores, register-valued wait threshold
    (schedule-time sim cannot constant-fold it).
  - k*rs fused via krep matmul instead of 16 per-tl tensor_scalars.
  - Copies/casts moved to ACT; stats chain shortened; single Newton iter.
"""
import sys
import numpy as np

sys.path.insert(0, "/opt/trn_rl_repo")

I, L, T, S = 128, 8, 128, 128
N_CORES = 8
TL = T // N_CORES
EPS = 1e-5
RS = float(1.0 / np.sqrt(np.float32(S)))
GC = 0.7978845608028654
GA = 0.044715
MAGIC = 0x5F3759DF

_cached = None


def _patch_topology():
    """No /dev/neuron* client-side: give the sim the static TRN2 NC map it
    needs to route remote DMA (the NEFF itself uses relative XOR routing)."""
    from concourse import libnrt
    base = (0, 1, 2, 3, 6, 7, 4, 5)

    def get_trn2_nc_mapping():
        return {(d, k): base[k] for d in range(16) for k in range(8)}

    def nc_to_real_nc(device_index, nc_index):
        return base[nc_index]

    def pnc_id_to_device_and_real_nc_index(core_id):
        return core_id // 8, base[core_id % 8]

    def get_device_id_to_routing_id_mapping():
        return {d: d for d in range(16)}

    libnrt.get_trn2_nc_mapping = get_trn2_nc_mapping
    libnrt.nc_to_real_nc = nc_to_real_nc
    libnrt.pnc_id_to_device_and_real_nc_index = pnc_id_to_device_and_real_nc_index
    libnrt.get_device_id_to_routing_id_mapping = get_device_id_to_routing_id_mapping
    for modname in ("concourse.bass_interp", "concourse.dge_state"):
        m = sys.modules.get(modname)
        if m is None:
            continue
        for fn in (nc_to_real_nc, pnc_id_to_device_and_real_nc_index,
                   get_device_id_to_routing_id_mapping):
            if hasattr(m, fn.__name__):
                setattr(m, fn.__name__, fn)


def _build():
    _patch_topology()
    from concourse import bacc, tile, mybir

    fp32 = mybir.dt.float32
    bf16 = mybir.dt.bfloat16
    int32 = mybir.dt.int32
    Exp = mybir.ActivationFunctionType.Exp
    Tanh = mybir.ActivationFunctionType.Tanh
    Copy = mybir.ActivationFunctionType.Copy
    Ident = mybir.ActivationFunctionType.Identity
    mul_op = mybir.AluOpType.mult
    add_op = mybir.AluOpType.add
    sub_op = mybir.AluOpType.subtract
    shr_op = mybir.AluOpType.arith_shift_right
    AxX = mybir.AxisListType.X

    nc = bacc.Bacc("TRN2", target_bir_lowering=False, debug=False,
                   enable_asserts=True, num_devices=N_CORES)

    tqkv_d = nc.dram_tensor("tqkv", [L, S, 3 * TL * S], bf16,
                            kind="ExternalInput").ap()  # [L,S,6144]: Q | per-tl k,v
    topo_wt_d = nc.dram_tensor("topo_wt", [L, S, 3 * S], fp32,
                               kind="ExternalInput").ap()
    small_d = nc.dram_tensor("small", [L, S, 72], fp32, kind="ExternalInput").ap()
    # small cols: 0:32 kvbias | 32:48 wmt | 48:64 mt | 64:67 topo_c
    #             | 67:70 topo_bp | 70 gamma | 71 beta
    bqr_d = nc.dram_tensor("bqr", [L, TL * S], bf16, kind="ExternalInput").ap()
    pre_d = nc.dram_tensor("pre", [S, 18], fp32, kind="ExternalInput").ap()
    sel_d = nc.dram_tensor("sel", [TL, S], fp32, kind="ExternalInput").ap()
    wbr_d = nc.dram_tensor("wbr", [1, L * TL], fp32, kind="ExternalInput").ap()
    thr_d = nc.dram_tensor("thr", [1, 8], int32, kind="ExternalInput").ap()
    ident_d = nc.dram_tensor("ident", [S, S], fp32, kind="ExternalInput").ap()
    magic_d = nc.dram_tensor("magic", [1, 2], int32, kind="ExternalInput").ap()
    out_d = nc.dram_tensor("out", [TL, 1], fp32, kind="ExternalOutput").ap()

    rsems = [nc.alloc_semaphore(f"rsem{b}") for b in range(L - 1)]
    lsem = nc.alloc_semaphore("lsem")
    bsem = nc.alloc_semaphore("bsem")
    gsem = nc.alloc_semaphore("gsem")

    with tile.TileContext(nc) as tc:
        with tc.tile_pool(name="wpool", bufs=3) as wpool, \
             tc.tile_pool(name="spool", bufs=3) as spool, \
             tc.tile_pool(name="fixed", bufs=1) as fixed, \
             tc.tile_pool(name="work", bufs=1) as work, \
             tc.tile_pool(name="ps_big", bufs=1, space="PSUM") as ps_big, \
             tc.tile_pool(name="ps_sm", bufs=1, space="PSUM") as ps_sm:

            pre = fixed.tile([S, 18], fp32)
            nc.gpsimd.dma_start(pre[:], pre_d)
            thr = fixed.tile([1, 8], int32)
            nc.gpsimd.dma_start(thr[:], thr_d)
            magic = fixed.tile([1, 2], int32)
            nc.gpsimd.dma_start(magic[:], magic_d)
            ident = fixed.tile([S, S], fp32)
            nc.gpsimd.dma_start(ident[:], ident_d)
            sel = fixed.tile([TL, S], fp32)
            nc.gpsimd.dma_start(sel[:], sel_d)
            wbr = fixed.tile([1, L * TL], fp32)
            nc.gpsimd.dma_start(wbr[:], wbr_d)
            ones_col = fixed.tile([S, 1], fp32)
            nc.vector.memset(ones_col[:], 1.0)
            ones_row = fixed.tile([1, S], fp32)
            nc.vector.memset(ones_row[:], 1.0)
            ones_row_bf = fixed.tile([1, S], bf16)
            nc.vector.memset(ones_row_bf[:], 1.0)
            ones_mat_bf = fixed.tile([S, S], bf16)
            nc.vector.memset(ones_mat_bf[:], 1.0)
            one_one = fixed.tile([1, 1], fp32)
            nc.vector.memset(one_one[:], 1.0)
            pvr_t = fixed.tile([S, 2], bf16)
            nc.vector.memset(pvr_t[:], 1.0)

            bsrc = [fixed.tile([S, 1], fp32, name=f"bsrc{b}") for b in range(L - 1)]
            v8s = [fixed.tile([S, 7], fp32, name=f"v8_{b}") for b in range(L - 1)]

            v_col = work.tile([S, 1], fp32)
            u_col = work.tile([S, 1], fp32)
            up_col = work.tile([S, 1], fp32)
            up_bf = work.tile([S, 1], bf16)
            Ub_bf = work.tile([S, S], bf16)
            sc = work.tile([1, 12], fp32)
            sci = sc[:].bitcast(int32)
            bc_sb = work.tile([S, 2], fp32)
            qkvt_c = work.tile([S, 3], fp32)
            qk_row = work.tile([1, 2 * S], fp32)
            Et_sb = work.tile([S, S], bf16)
            v_tmp = work.tile([S, 1], fp32)
            kvn = work.tile([S, 2 * TL], fp32)
            krsA = work.tile([S, TL // 2], fp32)
            krsB = work.tile([S, TL // 2], fp32)
            pvr_n = work.tile([S, 2 * TL], bf16)
            sc_sb = work.tile([S, TL * S // 2], bf16)
            E_A = work.tile([S, TL * S // 2], bf16)
            E_B = work.tile([S, TL * S // 2], bf16)
            rden = work.tile([S, TL], fp32)
            zp = work.tile([S, TL], fp32)
            afr = work.tile([S, 2 * TL], fp32)
            aff_sb = work.tile([TL, 1], fp32)

            scoresA = ps_big.tile([S, 1024], fp32)       # 2 banks (tls 0-7)
            scoresB = ps_big.tile([S, 1024], fp32)       # 2 banks (tls 8-15)
            smps = ps_sm.tile([S, 512], fp32)            # 1 bank
            kv_ps = smps[:, 0:32]
            pvn_ps = smps[:, 32:64]
            af_ps = smps[0:TL, 64:65]
            v128_ps = smps[:, 66:67]
            sv_ps = smps[0:1, 68:69]
            svv_ps = smps[0:1, 69:70]
            bc_ps = smps[:, 70:72]
            A_ps = smps[:, 72:75]
            pvt_ps = smps[:, 76:78]
            trq_ps = smps[0:1, 256:384]
            trk_ps = smps[0:1, 384:512]
            tsc_ps = scoresA[:, 0:128]  # free until the Q-bcast (strictly later)

            rthr_cm = nc.vector.register("rthr")
            rthr = rthr_cm.__enter__()
            nc.vector.reg_load(rthr, thr[0:1, 0:1])

            # Tracked WAW edge: reg_save writes a byte of v_col, so every
            # later v_col writer (incl. the sem-waiting reduce) orders after
            # the reg_load (register deps inside wait conditions are not
            # tracked by tile).
            nc.vector.reg_save(v_tmp[0:1, 0:1].bitcast(int32), rthr)

            def ts(out, in0, s1, op0, s2=None, op1=None, eng=None):
                e = eng or nc.vector
                if s2 is None:
                    e.tensor_scalar(out, in0, s1, None, op0)
                else:
                    e.tensor_scalar(out, in0, s1, s2, op0, op1)

            for b in range(L):
                # ---- weight prefetch (tqkv split in 4 so the gather trigger
                # never queues behind a >1us DMA) ----
                tqkv = wpool.tile([S, 3 * TL * S], bf16, tag="tqkv")
                for q in range(3):
                    nc.sync.dma_start(tqkv[:, q * 2048:(q + 1) * 2048],
                                      tqkv_d[b][:, q * 2048:(q + 1) * 2048])
                topo_wt = spool.tile([S, 3 * S], fp32, tag="topo_wt")
                nc.gpsimd.dma_start(topo_wt[:], topo_wt_d[b])
                small = spool.tile([S, 72], fp32, tag="small")
                nc.sync.dma_start(small[:], small_d[b])
                bqr = spool.tile([1, TL * S], bf16, tag="bqr")
                nc.sync.dma_start(bqr[:], bqr_d[b])
                kvbias = small[:, 0:32]
                wmt = small[:, 32:48]
                mt = small[:, 48:64]
                topo_c = small[:, 64:67]
                topo_bp = small[:, 67:70]
                gam = small[:, 70:71]
                bet = small[:, 71:72]

                # ---- desc-gen for THIS batch's end-of-batch broadcast (the
                # trigger at the end of this batch fires these 8 preps) ----
                if b < L - 1:
                    if b >= 2:
                        # SWDGE ring holds ~14 preps. Dummy write to bsrc[b]
                        # reading bsrc[b-2] (a declared output of trigger
                        # b-2): the preps' no-sync src edge then orders them
                        # after trigger b-2 on the in-order Pool queue, so
                        # ring entries are reclaimed before desc-gen.
                        ts(bsrc[b][0:1, 0:1], ones_row[0:1, 0:1],
                           bsrc[b - 2][0:1, 0:1], mul_op)
                    for k in range(1, N_CORES):
                        rd = [None] * 8
                        rd[k] = (0, k)
                        nc.gpsimd.remote_dma_broadcast(
                            v8s[b][:, k - 1:k], bsrc[b][:],
                            rsems[b], lsem, rdests=rd)

                # ---- acquire v ----
                if b == 0:
                    nc.vector.tensor_copy(v_col[:], pre[:, 0:1])
                else:
                    red = nc.vector.tensor_reduce(v_tmp[:], v8s[b - 1][:],
                                                  AxX, add_op)
                    red.wait_op(rsems[b - 1], rthr, "sem-ge")
                    nc.vector.tensor_add(v_col[:], v_tmp[:], bsrc[b - 1][:])

                # ---- topo qkv on raw v (PE, parallel with stats) ----
                for m in range(3):
                    nc.tensor.matmul(A_ps[:, m:m + 1],
                                     topo_wt[:, m * S:(m + 1) * S],
                                     v_col[:], start=True, stop=True)

                # ---- stats + Newton-1 rsqrt ----
                nc.tensor.matmul(sv_ps, ones_col[:], v_col[:], start=True, stop=True)
                nc.tensor.matmul(svv_ps, v_col[:], v_col[:], start=True, stop=True)
                ts(sc[:, 0:1], sv_ps, 1.0 / S, mul_op)
                ts(sc[:, 1:2], svv_ps, 1.0 / S, mul_op)
                nc.vector.scalar_tensor_tensor(sc[:, 3:4], sc[:, 0:1], sc[:, 0:1],
                                               sc[:, 1:2], mul_op, sub_op)
                ts(sc[:, 4:5], sc[:, 3:4], -1.0, mul_op, EPS, add_op)      # vpe
                ts(sc[:, 5:6], sc[:, 3:4], -0.5, mul_op, 0.5 * EPS, add_op)  # vh
                ts(sci[:, 8:9], sci[:, 4:5], 1, shr_op)
                nc.vector.tensor_sub(sci[:, 6:7], magic[:, 0:1], sci[:, 8:9])
                nc.vector.scalar_tensor_tensor(sc[:, 8:9], sc[:, 6:7], sc[:, 5:6],
                                               sc[:, 6:7], mul_op, mul_op)
                ts(sc[:, 8:9], sc[:, 8:9], -1.0, mul_op, 1.5, add_op)
                nc.vector.tensor_mul(sc[:, 6:7], sc[:, 6:7], sc[:, 8:9])   # rstd
                nc.vector.tensor_mul(sc[:, 7:8], sc[:, 6:7], sc[:, 0:1])   # mu*rstd
                nc.tensor.matmul(bc_ps, ones_row[:], sc[:, 6:8], start=True, stop=True)
                nc.scalar.activation(bc_sb[:], bc_ps, Copy)
                rstd_c = bc_sb[:, 0:1]
                murstd_c = bc_sb[:, 1:2]

                # ---- u = rstd*gamma*(v-mu) + beta  (ACT) ----
                grstd = work.tile([S, 1], fp32, tag="grstd")
                gmr = work.tile([S, 1], fp32, tag="gmr")
                boff = work.tile([S, 1], fp32, tag="boff")
                ts(grstd[:], gam, rstd_c, mul_op)
                ts(gmr[:], gam, murstd_c, mul_op)
                nc.vector.tensor_sub(boff[:], bet, gmr[:])
                nc.scalar.activation(u_col[:], v_col[:], Ident,
                                     bias=boff[:, 0:1], scale=grstd[:, 0:1])

                # ---- topo attention ----
                cm = work.tile([S, 3], fp32, tag="cm")
                ts(cm[:], topo_c, murstd_c, mul_op)
                nc.vector.scalar_tensor_tensor(qkvt_c[:], A_ps, rstd_c, cm[:],
                                               mul_op, sub_op)
                nc.vector.tensor_add(qkvt_c[:], qkvt_c[:], topo_bp)
                nc.tensor.transpose(trq_ps, qkvt_c[:, 0:1], ident[:])
                nc.tensor.transpose(trk_ps, qkvt_c[:, 1:2], ident[:])
                nc.vector.tensor_copy(qk_row[:], smps[0:1, 256:512])
                nc.tensor.matmul(tsc_ps, qk_row[0:1, S:2 * S],
                                 qk_row[0:1, 0:S], start=True, stop=True)
                nc.scalar.activation(Et_sb[:], tsc_ps, Exp, scale=RS)
                nc.scalar.activation(pvr_t[:, 0:1], qkvt_c[:, 2:3], Copy)
                nc.tensor.matmul(pvt_ps, Et_sb[:], pvr_t[:], start=True, stop=True)
                rd1 = work.tile([S, 1], fp32, tag="rd1")
                nc.vector.reciprocal(rd1[:], pvt_ps[:, 1:2])
                nc.vector.scalar_tensor_tensor(up_col[:], pvt_ps[:, 0:1],
                                               rd1[:, 0:1], u_col[:],
                                               mul_op, add_op)
                nc.scalar.activation(up_bf[:], up_col[:], Copy)
                ts(Ub_bf[:], ones_mat_bf[:], up_col[:, 0:1], mul_op)

                # ---- neuron k,v columns (PE; stationary-load not the cost) ----
                for tl in range(TL):
                    base = 2048 + tl * 256
                    nc.tensor.matmul(kv_ps[:, 2 * tl:2 * tl + 1],
                                     tqkv[:, base:base + S],
                                     up_bf[:], start=True, stop=True)
                    nc.tensor.matmul(kv_ps[:, 2 * tl + 1:2 * tl + 2],
                                     tqkv[:, base + S:base + 2 * S],
                                     up_bf[:], start=True, stop=True)
                nc.vector.tensor_add(kvn[:], kv_ps, kvbias)
                k2 = kvn[:].rearrange("p (t k) -> p t k", k=2)
                ts(krsA[:], k2[:, 0:8, 0], RS, mul_op)
                ts(krsB[:], k2[:, 8:16, 0], RS, mul_op)
                p2 = pvr_n[:].rearrange("p (t k) -> p t k", k=2)
                nc.vector.tensor_mul(p2[:, :, 0], k2[:, :, 1], mt)
                nc.scalar.activation(p2[:, :, 1], mt, Copy)

                # ---- Q broadcast (+ q bias) in PSUM, bf16; fused-ACT half
                # (tls 8-15, scoresB) first so its exps start earliest ----
                for half, dst in ((1, scoresB), (0, scoresA)):
                    for hb in range(2):
                        ds = slice(hb * 512, (hb + 1) * 512)
                        cs = slice(half * 1024 + hb * 512,
                                   half * 1024 + (hb + 1) * 512)
                        nc.tensor.matmul(dst[:, ds], Ub_bf[:], tqkv[:, cs],
                                         start=True, stop=False,
                                         skip_group_check=True)
                        nc.tensor.matmul(dst[:, ds], ones_row_bf[:],
                                         bqr[:, cs], start=False, stop=True,
                                         skip_group_check=True)

                # ---- k*rs scale + exp: fused-ACT for tls 8-15 (tiles B),
                # DVE-scale + two wide ACT exps for tls 0-7 (tiles A).
                # Disjoint tiles per half so the scheduler can't tie the
                # engines together with tile-granular edges. ----
                for j in range(8):
                    tl = 8 + j
                    nc.scalar.activation(E_B[:, j * S:(j + 1) * S],
                                         scoresB[:, j * S:(j + 1) * S],
                                         Exp, scale=krsB[:, j:j + 1])
                    nc.tensor.matmul(pvn_ps[:, 2 * tl:2 * tl + 2],
                                     E_B[:, j * S:(j + 1) * S],
                                     pvr_n[:, 2 * tl:2 * tl + 2],
                                     start=True, stop=True)
                for half in range(2):
                    for j in range(4):
                        tl = 4 * half + j
                        ts(sc_sb[:, tl * S:(tl + 1) * S],
                           scoresA[:, tl * S:(tl + 1) * S],
                           krsA[:, tl:tl + 1], mul_op)
                    gs = slice(half * 512, (half + 1) * 512)
                    nc.scalar.activation(E_A[:, gs], sc_sb[:, gs], Exp)
                    for j in range(4):
                        tl = 4 * half + j
                        nc.tensor.matmul(pvn_ps[:, 2 * tl:2 * tl + 2],
                                         E_A[:, tl * S:(tl + 1) * S],
                                         pvr_n[:, 2 * tl:2 * tl + 2],
                                         start=True, stop=True)

                pv2 = pvn_ps.rearrange("p (t k) -> p t k", k=2)
                nc.vector.reciprocal(rden[:], pv2[:, :, 1])
                nc.vector.tensor_mul(zp[:], pv2[:, :, 0], rden[:])

                # ---- aff = sum_i wmt*(zp + u') + wbias  (fp32) ----
                nc.vector.tensor_mul(afr[:, 0:TL], wmt, zp[:])
                ts(afr[:, TL:2 * TL], wmt, up_col[:, 0:1], mul_op)
                nc.tensor.matmul(af_ps, afr[:, 0:TL], ones_col[:],
                                 start=True, stop=False, skip_group_check=True)
                nc.tensor.matmul(af_ps, afr[:, TL:2 * TL], ones_col[:],
                                 start=False, stop=False, skip_group_check=True)
                nc.tensor.matmul(af_ps, wbr[:, b * TL:(b + 1) * TL], one_one[:],
                                 start=False, stop=True, skip_group_check=True)

                if b == L - 1:
                    nc.vector.tensor_copy(aff_sb[:], af_ps)
                    nc.sync.dma_start(out_d, aff_sb[:])
                else:
                    nc.vector.tensor_copy(aff_sb[:], af_ps)
                    nc.tensor.matmul(v128_ps, sel[:], aff_sb[:],
                                     start=True, stop=True)
                    # adaptive gelu (tanh approx), sel-placed column
                    g0 = pre[:, 2 + b:3 + b]
                    g1h = pre[:, 10 + b:11 + b]
                    xg = work.tile([S, 1], fp32, tag="xg")
                    s2t = work.tile([S, 1], fp32, tag="s2t")
                    t1 = work.tile([S, 1], fp32, tag="t1")
                    ts(xg[:], v128_ps, g0[:, 0:1], mul_op)
                    nc.vector.tensor_mul(s2t[:], xg[:], xg[:])
                    ts(t1[:], s2t[:], GA, mul_op, 1.0, add_op)
                    nc.vector.tensor_mul(t1[:], t1[:], xg[:])
                    nc.scalar.activation(t1[:], t1[:], Tanh, scale=GC)
                    nc.vector.scalar_tensor_tensor(t1[:], t1[:], 1.0, xg[:],
                                                   add_op, mul_op)
                    # WAW anchor for the next batch's reduce: without it the
                    # scheduler hoists the (sem-blocked) reduce to the head
                    # of the in-order DVE queue and wedges the whole engine.
                    # Reads t1 (not bsrc) to stay off the trigger's WAR path.
                    ts(v_tmp[0:1, 0:1], ones_row[0:1, 0:1],
                       t1[0:1, 0:1], mul_op)
                    ts(bsrc[b][:], t1[:], g1h[:, 0:1], mul_op)
                    nc.gpsimd.trigger_dma(count=None,
                                          signals_writable=[bsrc[b][:]])

    nc.compile()
    return nc


def _host_prep(x, W, mask, attn_t, attn_n, norm_params, ada):
    import ml_dtypes
    f32 = np.float32
    bf16 = ml_dtypes.bfloat16
    x, W, mask, attn_t, attn_n, norm_params, ada = (
        np.ascontiguousarray(np.asarray(a, f32))
        for a in (x, W, mask, attn_t, attn_n, norm_params, ada))
    gamma = norm_params[:, 0, :]
    beta = norm_params[:, 1, :]

    topo_w = attn_t[:, :, :, :S]
    topo_b = attn_t[:, :, :, S]
    topo_wg = topo_w * gamma[:, None, None, :]
    topo_wt_flat = np.ascontiguousarray(
        topo_wg.transpose(0, 3, 1, 2)).reshape(L, S, 3 * S)
    topo_c = topo_wg.sum(axis=3)
    topo_bp = np.einsum('lmis,ls->lmi', topo_w, beta) + topo_b

    wmat = W[:, :, :S] * mask
    wbias = W[:, :, S]

    pre = np.zeros((S, 18), f32)
    pre[:, 0] = x
    pre[:, 2:10] = ada[:, :, 0].T
    pre[:, 10:18] = (0.5 * ada[:, :, 1]).astype(f32).T

    ident = np.eye(S, dtype=f32)
    magic = np.array([[MAGIC, 0]], np.int32)
    thr = np.full((1, 8), 14, np.int32)
    
    in_maps = []
    for c in range(N_CORES):
        sl = slice(c * TL, (c + 1) * TL)
        an = attn_n[:, sl]
        anw = an[:, :, :, :, :S]                              # (L,TL,3,i,p)
        anb = an[:, :, :, :, S]                               # (L,TL,3,i)
        qpart = np.ascontiguousarray(
            anw[:, :, 0].transpose(0, 3, 1, 2)).reshape(L, S, TL * S)
        kvpart = np.ascontiguousarray(
            anw[:, :, 1:3].transpose(0, 4, 1, 2, 3)).reshape(L, S, TL * 2 * S)
        tqkv = np.concatenate([qpart, kvpart], axis=2).astype(bf16)
        small = np.zeros((L, S, 72), f32)
        kv = np.stack([anb[:, :, 1, :], anb[:, :, 2, :]], axis=2)  # (L,TL,2,i)
        small[:, :, 0:32] = kv.transpose(0, 3, 1, 2).reshape(L, S, 2 * TL)
        small[:, :, 32:48] = wmat[:, sl].transpose(0, 2, 1)
        small[:, :, 48:64] = mask[:, sl].transpose(0, 2, 1)
        small[:, :, 64:67] = topo_c.transpose(0, 2, 1)
        small[:, :, 67:70] = topo_bp.transpose(0, 2, 1)
        small[:, :, 70] = gamma
        small[:, :, 71] = beta
        bqr = np.ascontiguousarray(
            anb[:, :, 0, :].reshape(L, TL * S)).astype(bf16)
        selm = np.zeros((TL, S), f32)
        for j in range(TL):
            selm[j, c * TL + j] = 1.0
        wbr = np.ascontiguousarray(wbias[:, sl].reshape(1, L * TL))
        in_maps.append(dict(tqkv=tqkv, topo_wt=topo_wt_flat, small=small,
                            bqr=bqr, pre=pre, sel=selm, wbr=wbr,
                            thr=thr, ident=ident, magic=magic))
    return in_maps


def kernel(x, W, mask, attn_t, attn_n, attn_mask_n, norm_params, ada,
           span_ids, tb_ids):
    global _cached
    _patch_topology()
    from concourse import bass_utils
    if _cached is None:
        _cached = _build()
    nc = _cached
    in_maps = _host_prep(x, W, mask, attn_t, attn_n, norm_params, ada)
    res = bass_utils.run_bass_kernel_spmd(nc, in_maps, core_ids=list(range(N_CORES)))
    out = np.concatenate([res.results[c]["out"].reshape(TL) for c in range(N_CORES)])
    return out.astype(np.float32)


# revision 4
# speedup vs baseline: 1.0351x; 1.0351x over previous
"""v3 Trainium2 Bass kernel.

Scheme (per core c of 8, per topo batch b of 8):
  gather v (remote-DMA all-gather, NOT gpsimd collective) -> layernorm stats
  (Newton-1 rsqrt) -> topo self-attention on the span -> per-neuron
  self-attention for this core's TL=16 neurons (bf16 matmuls; k*rs scale
  built as a PE matmul krep = krsT @ dsel and applied as one DVE multiply
  per 512-col bank, then one ACT exp per bank) -> masked affine (fp32)
  -> adaptive gelu on the sel-placed [128,1] column -> remote_dma_broadcast
  to all 8 cores' SBUF (slot k -> tpb my^k), wait on per-batch remote sem.

v3 changes vs v2 (327.7us):
  - All heavy matmuls bf16 (1 cy/row vs 4 for fp32); affine stays fp32.
  - gpsimd AllGather (15us/call in the cost model) replaced by 8
    single-dest remote_dma_broadcast preps + one trigger per batch
    (~1-2us); per-batch remote semaphores, register-valued wait threshold
    (schedule-time sim cannot constant-fold it).
  - k*rs fused via krep matmul instead of 16 per-tl tensor_scalars.
  - Copies/casts moved to ACT; stats chain shortened; single Newton iter.
"""
import sys
import numpy as np

sys.path.insert(0, "/opt/trn_rl_repo")

I, L, T, S = 128, 8, 128, 128
N_CORES = 8
TL = T // N_CORES
EPS = 1e-5
RS = float(1.0 / np.sqrt(np.float32(S)))
GC = 0.7978845608028654
GA = 0.044715
MAGIC = 0x5F3759DF

_cached = None


def _patch_topology():
    """No /dev/neuron* client-side: give the sim the static TRN2 NC map it
    needs to route remote DMA (the NEFF itself uses relative XOR routing)."""
    from concourse import libnrt
    base = (0, 1, 2, 3, 6, 7, 4, 5)

    def get_trn2_nc_mapping():
        return {(d, k): base[k] for d in range(16) for k in range(8)}

    def nc_to_real_nc(device_index, nc_index):
        return base[nc_index]

    def pnc_id_to_device_and_real_nc_index(core_id):
        return core_id // 8, base[core_id % 8]

    def get_device_id_to_routing_id_mapping():
        return {d: d for d in range(16)}

    libnrt.get_trn2_nc_mapping = get_trn2_nc_mapping
    libnrt.nc_to_real_nc = nc_to_real_nc
    libnrt.pnc_id_to_device_and_real_nc_index = pnc_id_to_device_and_real_nc_index
    libnrt.get_device_id_to_routing_id_mapping = get_device_id_to_routing_id_mapping
    for modname in ("concourse.bass_interp", "concourse.dge_state"):
        m = sys.modules.get(modname)
        if m is None:
            continue
        for fn in (nc_to_real_nc, pnc_id_to_device_and_real_nc_index,
                   get_device_id_to_routing_id_mapping):
            if hasattr(m, fn.__name__):
                setattr(m, fn.__name__, fn)


def _build():
    _patch_topology()
    from concourse import bacc, tile, mybir

    fp32 = mybir.dt.float32
    bf16 = mybir.dt.bfloat16
    int32 = mybir.dt.int32
    Exp = mybir.ActivationFunctionType.Exp
    Tanh = mybir.ActivationFunctionType.Tanh
    Copy = mybir.ActivationFunctionType.Copy
    Ident = mybir.ActivationFunctionType.Identity
    mul_op = mybir.AluOpType.mult
    add_op = mybir.AluOpType.add
    sub_op = mybir.AluOpType.subtract
    shr_op = mybir.AluOpType.arith_shift_right
    AxX = mybir.AxisListType.X

    nc = bacc.Bacc("TRN2", target_bir_lowering=False, debug=False,
                   enable_asserts=True, num_devices=N_CORES)

    tqkv_d = nc.dram_tensor("tqkv", [L, S, 3 * TL * S + 3 * S], bf16,
                            kind="ExternalInput").ap()  # Q | per-tl k,v | topo qkv
    small_d = nc.dram_tensor("small", [L, S, 72], fp32, kind="ExternalInput").ap()
    # small cols: 0:32 kvbias | 32:48 wmt | 48:64 mt | 64:67 topo_c
    #             | 67:70 topo_bp | 70 gamma | 71 beta
    bqr_d = nc.dram_tensor("bqr", [L, TL * S], bf16, kind="ExternalInput").ap()
    pre_d = nc.dram_tensor("pre", [S, 18], fp32, kind="ExternalInput").ap()
    sel_d = nc.dram_tensor("sel", [TL, S], fp32, kind="ExternalInput").ap()
    wbr_d = nc.dram_tensor("wbr", [1, L * TL], fp32, kind="ExternalInput").ap()
    thr_d = nc.dram_tensor("thr", [1, 8], int32, kind="ExternalInput").ap()
    ident_d = nc.dram_tensor("ident", [S, S], fp32, kind="ExternalInput").ap()
    magic_d = nc.dram_tensor("magic", [1, 2], int32, kind="ExternalInput").ap()
    out_d = nc.dram_tensor("out", [TL, 1], fp32, kind="ExternalOutput").ap()

    rsems = [nc.alloc_semaphore(f"rsem{b}") for b in range(L - 1)]
    lsem = nc.alloc_semaphore("lsem")
    bsem = nc.alloc_semaphore("bsem")
    gsem = nc.alloc_semaphore("gsem")

    with tile.TileContext(nc) as tc:
        with tc.tile_pool(name="wpool", bufs=3) as wpool, \
             tc.tile_pool(name="spool", bufs=3) as spool, \
             tc.tile_pool(name="fixed", bufs=1) as fixed, \
             tc.tile_pool(name="work", bufs=1) as work, \
             tc.tile_pool(name="ps_big", bufs=1, space="PSUM") as ps_big, \
             tc.tile_pool(name="ps_sm", bufs=1, space="PSUM") as ps_sm:

            pre = fixed.tile([S, 18], fp32)
            nc.sync.dma_start(pre[:], pre_d)
            thr = fixed.tile([1, 8], int32)
            nc.sync.dma_start(thr[:], thr_d)
            magic = fixed.tile([1, 2], int32)
            nc.sync.dma_start(magic[:], magic_d)
            ident = fixed.tile([S, S], fp32)
            nc.scalar.dma_start(ident[:], ident_d)
            sel = fixed.tile([TL, S], fp32)
            nc.scalar.dma_start(sel[:], sel_d)
            wbr = fixed.tile([1, L * TL], fp32)
            nc.scalar.dma_start(wbr[:], wbr_d)
            ones_col = fixed.tile([S, 1], fp32)
            nc.vector.memset(ones_col[:], 1.0)
            ones_row = fixed.tile([1, S], fp32)
            nc.vector.memset(ones_row[:], 1.0)
            ones_row_bf = fixed.tile([1, S], bf16)
            nc.vector.memset(ones_row_bf[:], 1.0)
            ones_mat_bf = fixed.tile([S, S], bf16)
            nc.vector.memset(ones_mat_bf[:], 1.0)
            one_one = fixed.tile([1, 1], fp32)
            nc.vector.memset(one_one[:], 1.0)
            pvr_t = fixed.tile([S, 2], bf16)
            nc.vector.memset(pvr_t[:], 1.0)

            bsrc = [fixed.tile([S, 1], fp32, name=f"bsrc{b}") for b in range(L - 1)]
            v8s = [fixed.tile([S, 7], fp32, name=f"v8_{b}") for b in range(L - 1)]

            v_col = work.tile([S, 1], fp32)
            u_col = work.tile([S, 1], fp32)
            up_col = work.tile([S, 1], fp32)
            up_bf = work.tile([S, 1], bf16)
            v_bf = work.tile([S, 1], bf16)
            Ub_bf = work.tile([S, S], bf16)
            sc = work.tile([1, 12], fp32)
            sci = sc[:].bitcast(int32)
            bc_sb = work.tile([S, 2], fp32)
            qkvt_c = work.tile([S, 3], fp32)
            qk_row = work.tile([1, 2 * S], fp32)
            Et_sb = work.tile([S, S], bf16)
            v_tmp = work.tile([S, 1], fp32)
            kvn = work.tile([S, 2 * TL], fp32)
            krsA = work.tile([S, TL // 2], fp32)
            krsB = work.tile([S, TL // 2], fp32)
            pvr_n = work.tile([S, 2 * TL], bf16)
            sc_sb = work.tile([S, TL * S // 2], bf16)
            E_A = work.tile([S, TL * S // 2], bf16)
            E_B = work.tile([S, TL * S // 2], bf16)
            rden = work.tile([S, TL], fp32)
            zp = work.tile([S, TL], fp32)
            afr = work.tile([S, 2 * TL], fp32)
            aff_sb = work.tile([TL, 1], fp32)

            scoresA = ps_big.tile([S, 1024], fp32)       # 2 banks (tls 0-7)
            scoresB = ps_big.tile([S, 1024], fp32)       # 2 banks (tls 8-15)
            smps = ps_sm.tile([S, 512], fp32)            # 1 bank
            kv_ps = smps[:, 0:32]
            pvn_ps = smps[:, 32:64]
            af_ps = smps[0:TL, 64:65]
            v128_ps = smps[:, 66:67]
            sv_ps = smps[0:1, 68:69]
            svv_ps = smps[0:1, 69:70]
            bc_ps = smps[:, 70:72]
            A_ps = smps[:, 72:75]
            pvt_ps = smps[:, 76:78]
            trq_ps = smps[0:1, 256:384]
            trk_ps = smps[0:1, 384:512]
            tsc_ps = scoresA[:, 0:128]  # free until the Q-bcast (strictly later)

            rthr_cm = nc.vector.register("rthr")
            rthr = rthr_cm.__enter__()
            nc.vector.reg_load(rthr, thr[0:1, 0:1])

            # Tracked WAW edge: reg_save writes a byte of v_col, so every
            # later v_col writer (incl. the sem-waiting reduce) orders after
            # the reg_load (register deps inside wait conditions are not
            # tracked by tile).
            nc.vector.reg_save(v_tmp[0:1, 0:1].bitcast(int32), rthr)

            def ts(out, in0, s1, op0, s2=None, op1=None, eng=None):
                e = eng or nc.vector
                if s2 is None:
                    e.tensor_scalar(out, in0, s1, None, op0)
                else:
                    e.tensor_scalar(out, in0, s1, s2, op0, op1)

            for b in range(L):
                # ---- weight prefetch (tqkv split in 4 so the gather trigger
                # never queues behind a >1us DMA) ----
                tqkv = wpool.tile([S, 3 * TL * S + 3 * S], bf16, tag="tqkv")
                for q in range(3):
                    nc.sync.dma_start(tqkv[:, q * 2176:(q + 1) * 2176],
                                      tqkv_d[b][:, q * 2176:(q + 1) * 2176])
                small = spool.tile([S, 72], fp32, tag="small")
                nc.sync.dma_start(small[:], small_d[b])
                bqr = spool.tile([1, TL * S], bf16, tag="bqr")
                nc.sync.dma_start(bqr[:], bqr_d[b])
                kvbias = small[:, 0:32]
                wmt = small[:, 32:48]
                mt = small[:, 48:64]
                topo_c = small[:, 64:67]
                topo_bp = small[:, 67:70]
                gam = small[:, 70:71]
                bet = small[:, 71:72]

                # ---- desc-gen for THIS batch's end-of-batch broadcast (the
                # trigger at the end of this batch fires these 8 preps) ----
                if b < L - 1:
                    if b >= 2:
                        # SWDGE ring holds ~14 preps. Dummy write to bsrc[b]
                        # reading bsrc[b-2] (a declared output of trigger
                        # b-2): the preps' no-sync src edge then orders them
                        # after trigger b-2 on the in-order Pool queue, so
                        # ring entries are reclaimed before desc-gen.
                        ts(bsrc[b][0:1, 0:1], ones_row[0:1, 0:1],
                           bsrc[b - 2][0:1, 0:1], mul_op)
                    for k in range(1, N_CORES):
                        rd = [None] * 8
                        rd[k] = (0, k)
                        nc.gpsimd.remote_dma_broadcast(
                            v8s[b][:, k - 1:k], bsrc[b][:],
                            rsems[b], lsem, rdests=rd)

                # ---- acquire v ----
                if b == 0:
                    nc.vector.tensor_copy(v_col[:], pre[:, 0:1])
                else:
                    red = nc.vector.tensor_reduce(v_tmp[:], v8s[b - 1][:],
                                                  AxX, add_op)
                    red.wait_op(rsems[b - 1], rthr, "sem-ge")
                    nc.vector.tensor_add(v_col[:], v_tmp[:], bsrc[b - 1][:])

                # ---- topo qkv on raw v (PE, parallel with stats) ----
                nc.scalar.activation(v_bf[:], v_col[:], Copy)
                for m in range(3):
                    nc.tensor.matmul(A_ps[:, m:m + 1],
                                     tqkv[:, 6144 + m * S:6144 + (m + 1) * S],
                                     v_bf[:], start=True, stop=True)

                # ---- stats + Newton-1 rsqrt ----
                nc.tensor.matmul(sv_ps, ones_col[:], v_col[:], start=True, stop=True)
                nc.tensor.matmul(svv_ps, v_col[:], v_col[:], start=True, stop=True)
                ts(sc[:, 0:1], sv_ps, 1.0 / S, mul_op)
                ts(sc[:, 1:2], svv_ps, 1.0 / S, mul_op)
                nc.vector.scalar_tensor_tensor(sc[:, 3:4], sc[:, 0:1], sc[:, 0:1],
                                               sc[:, 1:2], mul_op, sub_op)
                ts(sc[:, 4:5], sc[:, 3:4], -1.0, mul_op, EPS, add_op)      # vpe
                ts(sc[:, 5:6], sc[:, 3:4], -0.5, mul_op, 0.5 * EPS, add_op)  # vh
                ts(sci[:, 8:9], sci[:, 4:5], 1, shr_op)
                nc.vector.tensor_sub(sci[:, 6:7], magic[:, 0:1], sci[:, 8:9])
                nc.vector.scalar_tensor_tensor(sc[:, 8:9], sc[:, 6:7], sc[:, 5:6],
                                               sc[:, 6:7], mul_op, mul_op)
                ts(sc[:, 8:9], sc[:, 8:9], -1.0, mul_op, 1.5, add_op)
                nc.vector.tensor_mul(sc[:, 6:7], sc[:, 6:7], sc[:, 8:9])   # rstd
                nc.vector.tensor_mul(sc[:, 7:8], sc[:, 6:7], sc[:, 0:1])   # mu*rstd
                nc.tensor.matmul(bc_ps, ones_row[:], sc[:, 6:8], start=True, stop=True)
                nc.scalar.activation(bc_sb[:], bc_ps, Copy)
                rstd_c = bc_sb[:, 0:1]
                murstd_c = bc_sb[:, 1:2]

                # ---- u = rstd*gamma*(v-mu) + beta  (ACT) ----
                grstd = work.tile([S, 1], fp32, tag="grstd")
                gmr = work.tile([S, 1], fp32, tag="gmr")
                boff = work.tile([S, 1], fp32, tag="boff")
                ts(grstd[:], gam, rstd_c, mul_op)
                ts(gmr[:], gam, murstd_c, mul_op)
                nc.vector.tensor_sub(boff[:], bet, gmr[:])
                nc.scalar.activation(u_col[:], v_col[:], Ident,
                                     bias=boff[:, 0:1], scale=grstd[:, 0:1])

                # ---- topo attention ----
                cm = work.tile([S, 3], fp32, tag="cm")
                ts(cm[:], topo_c, murstd_c, mul_op)
                nc.vector.scalar_tensor_tensor(qkvt_c[:], A_ps, rstd_c, cm[:],
                                               mul_op, sub_op)
                nc.vector.tensor_add(qkvt_c[:], qkvt_c[:], topo_bp)
                nc.tensor.transpose(trq_ps, qkvt_c[:, 0:1], ident[:])
                nc.tensor.transpose(trk_ps, qkvt_c[:, 1:2], ident[:])
                nc.vector.tensor_copy(qk_row[:], smps[0:1, 256:512])
                nc.tensor.matmul(tsc_ps, qk_row[0:1, S:2 * S],
                                 qk_row[0:1, 0:S], start=True, stop=True)
                nc.scalar.activation(Et_sb[:], tsc_ps, Exp, scale=RS)
                nc.scalar.activation(pvr_t[:, 0:1], qkvt_c[:, 2:3], Copy)
                nc.tensor.matmul(pvt_ps, Et_sb[:], pvr_t[:], start=True, stop=True)
                rd1 = work.tile([S, 1], fp32, tag="rd1")
                nc.vector.reciprocal(rd1[:], pvt_ps[:, 1:2])
                nc.vector.scalar_tensor_tensor(up_col[:], pvt_ps[:, 0:1],
                                               rd1[:, 0:1], u_col[:],
                                               mul_op, add_op)
                nc.scalar.activation(up_bf[:], up_col[:], Copy)
                ts(Ub_bf[:], ones_mat_bf[:], up_col[:, 0:1], mul_op)

                # ---- neuron k,v columns (PE; stationary-load not the cost) ----
                for tl in range(TL):
                    base = 2048 + tl * 256
                    nc.tensor.matmul(kv_ps[:, 2 * tl:2 * tl + 1],
                                     tqkv[:, base:base + S],
                                     up_bf[:], start=True, stop=True)
                    nc.tensor.matmul(kv_ps[:, 2 * tl + 1:2 * tl + 2],
                                     tqkv[:, base + S:base + 2 * S],
                                     up_bf[:], start=True, stop=True)
                nc.vector.tensor_add(kvn[:], kv_ps, kvbias)
                k2 = kvn[:].rearrange("p (t k) -> p t k", k=2)
                ts(krsA[:], k2[:, 0:8, 0], RS, mul_op)
                ts(krsB[:], k2[:, 8:16, 0], RS, mul_op)
                p2 = pvr_n[:].rearrange("p (t k) -> p t k", k=2)
                nc.vector.tensor_mul(p2[:, :, 0], k2[:, :, 1], mt)
                nc.scalar.activation(p2[:, :, 1], mt, Copy)

                # ---- Q broadcast (+ q bias) in PSUM, bf16; fused-ACT half
                # (tls 8-15, scoresB) first so its exps start earliest ----
                for half, dst in ((1, scoresB), (0, scoresA)):
                    for hb in range(2):
                        ds = slice(hb * 512, (hb + 1) * 512)
                        cs = slice(half * 1024 + hb * 512,
                                   half * 1024 + (hb + 1) * 512)
                        nc.tensor.matmul(dst[:, ds], Ub_bf[:], tqkv[:, cs],
                                         start=True, stop=False,
                                         skip_group_check=True)
                        nc.tensor.matmul(dst[:, ds], ones_row_bf[:],
                                         bqr[:, cs], start=False, stop=True,
                                         skip_group_check=True)

                # ---- k*rs scale + exp: fused-ACT for tls 8-15 (tiles B),
                # DVE-scale + two wide ACT exps for tls 0-7 (tiles A).
                # Disjoint tiles per half so the scheduler can't tie the
                # engines together with tile-granular edges. ----
                for j in range(8):
                    tl = 8 + j
                    nc.scalar.activation(E_B[:, j * S:(j + 1) * S],
                                         scoresB[:, j * S:(j + 1) * S],
                                         Exp, scale=krsB[:, j:j + 1])
                    nc.tensor.matmul(pvn_ps[:, 2 * tl:2 * tl + 2],
                                     E_B[:, j * S:(j + 1) * S],
                                     pvr_n[:, 2 * tl:2 * tl + 2],
                                     start=True, stop=True)
                for half in range(2):
                    for j in range(4):
                        tl = 4 * half + j
                        ts(sc_sb[:, tl * S:(tl + 1) * S],
                           scoresA[:, tl * S:(tl + 1) * S],
                           krsA[:, tl:tl + 1], mul_op)
                    gs = slice(half * 512, (half + 1) * 512)
                    nc.scalar.activation(E_A[:, gs], sc_sb[:, gs], Exp)
                    for j in range(4):
                        tl = 4 * half + j
                        nc.tensor.matmul(pvn_ps[:, 2 * tl:2 * tl + 2],
                                         E_A[:, tl * S:(tl + 1) * S],
                                         pvr_n[:, 2 * tl:2 * tl + 2],
                                         start=True, stop=True)

                pv2 = pvn_ps.rearrange("p (t k) -> p t k", k=2)
                nc.vector.reciprocal(rden[:], pv2[:, :, 1])
                nc.vector.tensor_mul(zp[:], pv2[:, :, 0], rden[:])

                # ---- aff = sum_i wmt*(zp + u') + wbias  (fp32) ----
                nc.vector.tensor_mul(afr[:, 0:TL], wmt, zp[:])
                ts(afr[:, TL:2 * TL], wmt, up_col[:, 0:1], mul_op)
                nc.tensor.matmul(af_ps, afr[:, 0:TL], ones_col[:],
                                 start=True, stop=False, skip_group_check=True)
                nc.tensor.matmul(af_ps, afr[:, TL:2 * TL], ones_col[:],
                                 start=False, stop=False, skip_group_check=True)
                nc.tensor.matmul(af_ps, wbr[:, b * TL:(b + 1) * TL], one_one[:],
                                 start=False, stop=True, skip_group_check=True)

                if b == L - 1:
                    nc.vector.tensor_copy(aff_sb[:], af_ps)
                    nc.sync.dma_start(out_d, aff_sb[:])
                else:
                    nc.vector.tensor_copy(aff_sb[:], af_ps)
                    nc.tensor.matmul(v128_ps, sel[:], aff_sb[:],
                                     start=True, stop=True)
                    # adaptive gelu (tanh approx), sel-placed column
                    g0 = pre[:, 2 + b:3 + b]
                    g1h = pre[:, 10 + b:11 + b]
                    xg = work.tile([S, 1], fp32, tag="xg")
                    s2t = work.tile([S, 1], fp32, tag="s2t")
                    t1 = work.tile([S, 1], fp32, tag="t1")
                    ts(xg[:], v128_ps, g0[:, 0:1], mul_op)
                    nc.vector.tensor_mul(s2t[:], xg[:], xg[:])
                    ts(t1[:], s2t[:], GA, mul_op, 1.0, add_op)
                    nc.vector.tensor_mul(t1[:], t1[:], xg[:])
                    nc.scalar.activation(t1[:], t1[:], Tanh, scale=GC)
                    nc.vector.scalar_tensor_tensor(t1[:], t1[:], 1.0, xg[:],
                                                   add_op, mul_op)
                    # WAW anchor for the next batch's reduce: without it the
                    # scheduler hoists the (sem-blocked) reduce to the head
                    # of the in-order DVE queue and wedges the whole engine.
                    # Reads t1 (not bsrc) to stay off the trigger's WAR path.
                    ts(v_tmp[0:1, 0:1], ones_row[0:1, 0:1],
                       t1[0:1, 0:1], mul_op)
                    ts(bsrc[b][:], t1[:], g1h[:, 0:1], mul_op)
                    nc.gpsimd.trigger_dma(count=None,
                                          signals_writable=[bsrc[b][:]])

    nc.compile()
    return nc


def _host_prep(x, W, mask, attn_t, attn_n, norm_params, ada):
    import ml_dtypes
    f32 = np.float32
    bf16 = ml_dtypes.bfloat16
    x, W, mask, attn_t, attn_n, norm_params, ada = (
        np.ascontiguousarray(np.asarray(a, f32))
        for a in (x, W, mask, attn_t, attn_n, norm_params, ada))
    gamma = norm_params[:, 0, :]
    beta = norm_params[:, 1, :]

    topo_w = attn_t[:, :, :, :S]
    topo_b = attn_t[:, :, :, S]
    topo_wg = topo_w * gamma[:, None, None, :]
    topo_wt_flat = np.ascontiguousarray(
        topo_wg.transpose(0, 3, 1, 2)).reshape(L, S, 3 * S)
    topo_c = topo_wg.sum(axis=3)
    topo_bp = np.einsum('lmis,ls->lmi', topo_w, beta) + topo_b

    wmat = W[:, :, :S] * mask
    wbias = W[:, :, S]

    pre = np.zeros((S, 18), f32)
    pre[:, 0] = x
    pre[:, 2:10] = ada[:, :, 0].T
    pre[:, 10:18] = (0.5 * ada[:, :, 1]).astype(f32).T

    ident = np.eye(S, dtype=f32)
    magic = np.array([[MAGIC, 0]], np.int32)
    thr = np.full((1, 8), 14, np.int32)
    
    in_maps = []
    for c in range(N_CORES):
        sl = slice(c * TL, (c + 1) * TL)
        an = attn_n[:, sl]
        anw = an[:, :, :, :, :S]                              # (L,TL,3,i,p)
        anb = an[:, :, :, :, S]                               # (L,TL,3,i)
        qpart = np.ascontiguousarray(
            anw[:, :, 0].transpose(0, 3, 1, 2)).reshape(L, S, TL * S)
        kvpart = np.ascontiguousarray(
            anw[:, :, 1:3].transpose(0, 4, 1, 2, 3)).reshape(L, S, TL * 2 * S)
        tqkv = np.concatenate([qpart, kvpart,
                               topo_wt_flat], axis=2).astype(bf16)
        small = np.zeros((L, S, 72), f32)
        kv = np.stack([anb[:, :, 1, :], anb[:, :, 2, :]], axis=2)  # (L,TL,2,i)
        small[:, :, 0:32] = kv.transpose(0, 3, 1, 2).reshape(L, S, 2 * TL)
        small[:, :, 32:48] = wmat[:, sl].transpose(0, 2, 1)
        small[:, :, 48:64] = mask[:, sl].transpose(0, 2, 1)
        small[:, :, 64:67] = topo_c.transpose(0, 2, 1)
        small[:, :, 67:70] = topo_bp.transpose(0, 2, 1)
        small[:, :, 70] = gamma
        small[:, :, 71] = beta
        bqr = np.ascontiguousarray(
            anb[:, :, 0, :].reshape(L, TL * S)).astype(bf16)
        selm = np.zeros((TL, S), f32)
        for j in range(TL):
            selm[j, c * TL + j] = 1.0
        wbr = np.ascontiguousarray(wbias[:, sl].reshape(1, L * TL))
        in_maps.append(dict(tqkv=tqkv, small=small,
                            bqr=bqr, pre=pre, sel=selm, wbr=wbr,
                            thr=thr, ident=ident, magic=magic))
    return in_maps


def kernel(x, W, mask, attn_t, attn_n, attn_mask_n, norm_params, ada,
           span_ids, tb_ids):
    global _cached
    _patch_topology()
    from concourse import bass_utils
    if _cached is None:
        _cached = _build()
    nc = _cached
    in_maps = _host_prep(x, W, mask, attn_t, attn_n, norm_params, ada)
    res = bass_utils.run_bass_kernel_spmd(nc, in_maps, core_ids=list(range(N_CORES)))
    out = np.concatenate([res.results[c]["out"].reshape(TL) for c in range(N_CORES)])
    return out.astype(np.float32)


# revision 5
# speedup vs baseline: 1.0699x; 1.0337x over previous
"""v3 Trainium2 Bass kernel.

Scheme (per core c of 8, per topo batch b of 8):
  gather v (remote-DMA all-gather, NOT gpsimd collective) -> layernorm stats
  (Newton-1 rsqrt) -> topo self-attention on the span -> per-neuron
  self-attention for this core's TL=16 neurons (bf16 matmuls; k*rs scale
  built as a PE matmul krep = krsT @ dsel and applied as one DVE multiply
  per 512-col bank, then one ACT exp per bank) -> masked affine (fp32)
  -> adaptive gelu on the sel-placed [128,1] column -> remote_dma_broadcast
  to all 8 cores' SBUF (slot k -> tpb my^k), wait on per-batch remote sem.

v3 changes vs v2 (327.7us):
  - All heavy matmuls bf16 (1 cy/row vs 4 for fp32); affine stays fp32.
  - gpsimd AllGather (15us/call in the cost model) replaced by 8
    single-dest remote_dma_broadcast preps + one trigger per batch
    (~1-2us); per-batch remote semaphores, register-valued wait threshold
    (schedule-time sim cannot constant-fold it).
  - k*rs fused via krep matmul instead of 16 per-tl tensor_scalars.
  - Copies/casts moved to ACT; stats chain shortened; single Newton iter.
"""
import sys
import numpy as np

sys.path.insert(0, "/opt/trn_rl_repo")

I, L, T, S = 128, 8, 128, 128
N_CORES = 8
TL = T // N_CORES
EPS = 1e-5
RS = float(1.0 / np.sqrt(np.float32(S)))
GC = 0.7978845608028654
GA = 0.044715
MAGIC = 0x5F3759DF

_cached = None


def _patch_topology():
    """No /dev/neuron* client-side: give the sim the static TRN2 NC map it
    needs to route remote DMA (the NEFF itself uses relative XOR routing)."""
    from concourse import libnrt
    base = (0, 1, 2, 3, 6, 7, 4, 5)

    def get_trn2_nc_mapping():
        return {(d, k): base[k] for d in range(16) for k in range(8)}

    def nc_to_real_nc(device_index, nc_index):
        return base[nc_index]

    def pnc_id_to_device_and_real_nc_index(core_id):
        return core_id // 8, base[core_id % 8]

    def get_device_id_to_routing_id_mapping():
        return {d: d for d in range(16)}

    libnrt.get_trn2_nc_mapping = get_trn2_nc_mapping
    libnrt.nc_to_real_nc = nc_to_real_nc
    libnrt.pnc_id_to_device_and_real_nc_index = pnc_id_to_device_and_real_nc_index
    libnrt.get_device_id_to_routing_id_mapping = get_device_id_to_routing_id_mapping
    for modname in ("concourse.bass_interp", "concourse.dge_state"):
        m = sys.modules.get(modname)
        if m is None:
            continue
        for fn in (nc_to_real_nc, pnc_id_to_device_and_real_nc_index,
                   get_device_id_to_routing_id_mapping):
            if hasattr(m, fn.__name__):
                setattr(m, fn.__name__, fn)


def _build():
    _patch_topology()
    from concourse import bacc, tile, mybir

    fp32 = mybir.dt.float32
    bf16 = mybir.dt.bfloat16
    int32 = mybir.dt.int32
    Exp = mybir.ActivationFunctionType.Exp
    Tanh = mybir.ActivationFunctionType.Tanh
    Copy = mybir.ActivationFunctionType.Copy
    Ident = mybir.ActivationFunctionType.Identity
    mul_op = mybir.AluOpType.mult
    add_op = mybir.AluOpType.add
    sub_op = mybir.AluOpType.subtract
    shr_op = mybir.AluOpType.arith_shift_right
    AxX = mybir.AxisListType.X

    nc = bacc.Bacc("TRN2", target_bir_lowering=False, debug=False,
                   enable_asserts=True, num_devices=N_CORES)

    tqkv_d = nc.dram_tensor("tqkv", [L, S, 6576], bf16,
                            kind="ExternalInput").ap()  # Q | k,v | topo | kvbias,mt
    bqr_d = nc.dram_tensor("bqr", [L, TL * S], bf16, kind="ExternalInput").ap()
    trow_d = nc.dram_tensor("trow", [1, L * 4 * S], bf16, kind="ExternalInput").ap()
    pre_d = nc.dram_tensor("pre", [S, 178], fp32, kind="ExternalInput").ap()
    sel_d = nc.dram_tensor("sel", [TL, S], fp32, kind="ExternalInput").ap()
    wbr_d = nc.dram_tensor("wbr", [1, L * TL], fp32, kind="ExternalInput").ap()
    thr_d = nc.dram_tensor("thr", [1, 8], int32, kind="ExternalInput").ap()
    magic_d = nc.dram_tensor("magic", [1, 2], int32, kind="ExternalInput").ap()
    out_d = nc.dram_tensor("out", [TL, 1], fp32, kind="ExternalOutput").ap()

    rsems = [nc.alloc_semaphore(f"rsem{b}") for b in range(L - 1)]
    lsem = nc.alloc_semaphore("lsem")
    bsem = nc.alloc_semaphore("bsem")
    gsem = nc.alloc_semaphore("gsem")

    with tile.TileContext(nc) as tc:
        with tc.tile_pool(name="wpool", bufs=3) as wpool, \
             tc.tile_pool(name="spool", bufs=3) as spool, \
             tc.tile_pool(name="fixed", bufs=1) as fixed, \
             tc.tile_pool(name="work", bufs=1) as work, \
             tc.tile_pool(name="ps_big", bufs=1, space="PSUM") as ps_big, \
             tc.tile_pool(name="ps_sm", bufs=1, space="PSUM") as ps_sm, \
             tc.tile_pool(name="ps_tp", bufs=1, space="PSUM") as ps_tp:

            pre = fixed.tile([S, 178], fp32)
            nc.sync.dma_start(pre[:], pre_d)
            magic = fixed.tile([1, 2], int32)
            nc.scalar.dma_start(magic[:], magic_d)
            trow = fixed.tile([1, L * 4 * S], bf16)
            nc.scalar.dma_start(trow[:], trow_d)
            thr = fixed.tile([1, 8], int32)
            sel = fixed.tile([TL, S], fp32)
            wbr = fixed.tile([1, L * TL], fp32)
            ones_col = fixed.tile([S, 1], fp32)
            nc.vector.memset(ones_col[:], 1.0)
            ones_row = fixed.tile([1, S], fp32)
            nc.vector.memset(ones_row[:], 1.0)
            ones_row_bf = fixed.tile([1, S], bf16)
            nc.vector.memset(ones_row_bf[:], 1.0)
            ones_mat_bf = fixed.tile([S, S], bf16)
            nc.vector.memset(ones_mat_bf[:], 1.0)
            one_one = fixed.tile([1, 1], fp32)
            nc.vector.memset(one_one[:], 1.0)
            pvr_t = fixed.tile([S, 2], bf16)
            nc.vector.memset(pvr_t[:], 1.0)

            bsrc = [fixed.tile([S, 1], fp32, name=f"bsrc{b}") for b in range(L - 1)]
            v8s = [fixed.tile([S, 7], fp32, name=f"v8_{b}") for b in range(L - 1)]

            v_col = work.tile([S, 1], fp32)
            u_col = work.tile([S, 1], fp32)
            up_col = work.tile([S, 1], fp32)
            up_bf = work.tile([S, 1], bf16)
            v_bf = work.tile([S, 1], bf16)
            Ub_bf = work.tile([S, S], bf16)
            sc = work.tile([1, 12], fp32)
            sci = sc[:].bitcast(int32)
            bc_sb = work.tile([S, 2], fp32)
            qkvt_c = work.tile([S, 3], fp32)
            qk_row = work.tile([1, 2 * S], fp32)
            cm2_row = work.tile([1, 2 * S], fp32)
            cmv = work.tile([S, 1], fp32)
            qkvt_v = work.tile([S, 1], fp32)
            Et_sb = work.tile([S, S], bf16)
            v_tmp = work.tile([S, 1], fp32)
            kvn = work.tile([S, 2 * TL], fp32)
            krsA = work.tile([S, TL // 2], fp32)
            krsB = work.tile([S, TL // 2], fp32)
            pvr_n = work.tile([S, 2 * TL], bf16)
            sc_sb = work.tile([S, TL * S // 2], bf16)
            E_A = work.tile([S, TL * S // 2], bf16)
            E_B = work.tile([S, TL * S // 2], bf16)
            rden = work.tile([S, TL], fp32)
            zp = work.tile([S, TL], fp32)
            afr = work.tile([S, 2 * TL], fp32)
            aff_sb = work.tile([TL, 1], fp32)

            scoresA = ps_big.tile([S, 1024], fp32)       # 2 banks (tls 0-7)
            scoresB = ps_big.tile([S, 1024], fp32)       # 2 banks (tls 8-15)
            smps = ps_sm.tile([S, 512], fp32)            # 1 bank
            kv_ps = smps[:, 0:32]
            pvn_ps = smps[:, 32:64]
            af_ps = smps[0:TL, 64:65]
            v128_ps = smps[:, 66:67]
            sv_ps = smps[0:1, 68:69]
            svv_ps = smps[0:1, 69:70]
            bc_ps = smps[:, 70:72]
            A_ps = smps[:, 72:75]
            pvt_ps = smps[:, 76:78]
            tpps = ps_tp.tile([S, 512], fp32)            # 1 bank (topo stage)
            qk0_ps = tpps[0:1, 128:384]
            tsc_ps = tpps[:, 0:128]

            rthr_cm = nc.vector.register("rthr")
            rthr = rthr_cm.__enter__()
            nc.vector.reg_load(rthr, thr[0:1, 0:1])

            # Tracked WAW edge: reg_save writes a byte of v_col, so every
            # later v_col writer (incl. the sem-waiting reduce) orders after
            # the reg_load (register deps inside wait conditions are not
            # tracked by tile).
            nc.vector.reg_save(v_tmp[0:1, 0:1].bitcast(int32), rthr)

            def ts(out, in0, s1, op0, s2=None, op1=None, eng=None):
                e = eng or nc.vector
                if s2 is None:
                    e.tensor_scalar(out, in0, s1, None, op0)
                else:
                    e.tensor_scalar(out, in0, s1, s2, op0, op1)

            for b in range(L):
                # ---- weight prefetch (tqkv split in 4 so the gather trigger
                # never queues behind a >1us DMA) ----
                tqkv = wpool.tile([S, 6576], bf16, tag="tqkv")
                bqr = spool.tile([1, TL * S], bf16, tag="bqr")
                if b > 0:
                    nc.sync.dma_start(bqr[:], bqr_d[b])
                for q in ((2, 1, 0) if b == 0 else range(3)):
                    nc.sync.dma_start(tqkv[:, q * 2192:(q + 1) * 2192],
                                      tqkv_d[b][:, q * 2192:(q + 1) * 2192])
                if b == 0:
                    nc.sync.dma_start(thr[:], thr_d)
                    nc.sync.dma_start(bqr[:], bqr_d[b])
                    # Tracked WAW edge: reg_save writes a byte of v_tmp, so
                    # every later v_tmp writer (incl. the sem-waiting reduce)
                    # orders after the reg_load (register deps inside wait
                    # conditions are not tracked by tile).
                    nc.vector.reg_load(rthr, thr[0:1, 0:1])
                    nc.vector.reg_save(v_tmp[0:1, 0:1].bitcast(int32), rthr)
                    # Tracked WAW edge: reg_save writes a byte of v_tmp, so
                    # every later v_tmp writer (incl. the sem-waiting reduce)
                    # orders after the reg_load (register deps inside wait
                    # conditions are not tracked by tile).
                    nc.vector.reg_load(rthr, thr[0:1, 0:1])
                    nc.vector.reg_save(v_tmp[0:1, 0:1].bitcast(int32), rthr)
                if b == 0:
                    # deferred: keeps the early ACT queue clear for batch 0;
                    # these land ~12us, first use ~15us (batch-0 tail)
                    nc.scalar.dma_start(sel[:], sel_d)
                    nc.scalar.dma_start(wbr[:], wbr_d)
                kvbias = tqkv[:, 6528:6560]
                mt = tqkv[:, 6560:6576]
                wmt = pre[:, 18 + 16 * b:18 + 16 * (b + 1)]
                gam = pre[:, 146 + b:147 + b]
                bet = pre[:, 154 + b:155 + b]

                # ---- desc-gen for THIS batch's end-of-batch broadcast (the
                # trigger at the end of this batch fires these 8 preps) ----
                if b < L - 1:
                    if b >= 2:
                        # SWDGE ring holds ~14 preps. Dummy write to bsrc[b]
                        # reading bsrc[b-2] (a declared output of trigger
                        # b-2): the preps' no-sync src edge then orders them
                        # after trigger b-2 on the in-order Pool queue, so
                        # ring entries are reclaimed before desc-gen.
                        ts(bsrc[b][0:1, 0:1], ones_row[0:1, 0:1],
                           bsrc[b - 2][0:1, 0:1], mul_op)
                    for k in range(1, N_CORES):
                        rd = [None] * 8
                        rd[k] = (0, k)
                        nc.gpsimd.remote_dma_broadcast(
                            v8s[b][:, k - 1:k], bsrc[b][:],
                            rsems[b], lsem, rdests=rd)

                # ---- acquire v ----
                if b == 0:
                    nc.vector.tensor_copy(v_col[:], pre[:, 0:1])
                else:
                    red = nc.vector.tensor_reduce(v_tmp[:], v8s[b - 1][:],
                                                  AxX, add_op)
                    red.wait_op(rsems[b - 1], rthr, "sem-ge")
                    nc.vector.tensor_add(v_col[:], v_tmp[:], bsrc[b - 1][:])

                # ---- topo qkv on raw v (PE, parallel with stats) ----
                nc.vector.tensor_copy(v_bf[:], v_col[:])
                nc.tensor.matmul(qk0_ps, v_bf[:], tqkv[:, 6144:6144 + 2 * S],
                                 start=True, stop=True)
                nc.tensor.matmul(A_ps[:, 2:3],
                                 tqkv[:, 6144 + 2 * S:6144 + 3 * S],
                                 v_bf[:], start=True, stop=True)

                # ---- stats + Newton-1 rsqrt ----
                nc.tensor.matmul(sv_ps, ones_col[:], v_col[:], start=True, stop=True)
                nc.tensor.matmul(svv_ps, v_col[:], v_col[:], start=True, stop=True)
                ts(sc[:, 0:1], sv_ps, 1.0 / S, mul_op)
                ts(sc[:, 1:2], svv_ps, 1.0 / S, mul_op)
                nc.vector.scalar_tensor_tensor(sc[:, 3:4], sc[:, 0:1], sc[:, 0:1],
                                               sc[:, 1:2], mul_op, sub_op)
                ts(sc[:, 4:5], sc[:, 3:4], -1.0, mul_op, EPS, add_op)      # vpe
                ts(sc[:, 5:6], sc[:, 3:4], -0.5, mul_op, 0.5 * EPS, add_op)  # vh
                ts(sci[:, 8:9], sci[:, 4:5], 1, shr_op)
                nc.vector.tensor_sub(sci[:, 6:7], magic[:, 0:1], sci[:, 8:9])
                nc.vector.scalar_tensor_tensor(sc[:, 8:9], sc[:, 6:7], sc[:, 5:6],
                                               sc[:, 6:7], mul_op, mul_op)
                ts(sc[:, 8:9], sc[:, 8:9], -1.0, mul_op, 1.5, add_op)
                nc.vector.tensor_mul(sc[:, 6:7], sc[:, 6:7], sc[:, 8:9])   # rstd
                nc.vector.tensor_mul(sc[:, 7:8], sc[:, 6:7], sc[:, 0:1])   # mu*rstd
                nc.tensor.matmul(bc_ps, ones_row[:], sc[:, 6:8], start=True, stop=True)
                nc.scalar.activation(bc_sb[:], bc_ps, Copy)
                rstd_c = bc_sb[:, 0:1]
                murstd_c = bc_sb[:, 1:2]

                # ---- u = rstd*gamma*(v-mu) + beta  (ACT) ----
                grstd = work.tile([S, 1], fp32, tag="grstd")
                gmr = work.tile([S, 1], fp32, tag="gmr")
                boff = work.tile([S, 1], fp32, tag="boff")
                ts(grstd[:], gam, rstd_c, mul_op)
                ts(gmr[:], gam, murstd_c, mul_op)
                nc.vector.tensor_sub(boff[:], bet, gmr[:])
                nc.scalar.activation(u_col[:], v_col[:], Ident,
                                     bias=boff[:, 0:1], scale=grstd[:, 0:1])

                # ---- topo attention: q,k corrected in row space (the
                # scalars live at partition 0 in sc, no broadcast needed) ----
                tc_row = trow[0:1, b * 512:b * 512 + 2 * S]
                bp_row = trow[0:1, b * 512 + 2 * S:(b + 1) * 512]
                nc.vector.scalar_tensor_tensor(cm2_row[:], tc_row, sc[:, 7:8],
                                               bp_row, mul_op, sub_op)
                nc.vector.scalar_tensor_tensor(qk_row[:], qk0_ps, sc[:, 6:7],
                                               cm2_row[:], mul_op, sub_op)
                nc.tensor.matmul(tsc_ps, qk_row[0:1, S:2 * S],
                                 qk_row[0:1, 0:S], start=True, stop=True)
                nc.vector.scalar_tensor_tensor(cmv[:], pre[:, 162 + b:163 + b],
                                               murstd_c[:, 0:1],
                                               pre[:, 170 + b:171 + b],
                                               mul_op, sub_op)
                nc.vector.scalar_tensor_tensor(qkvt_v[:], A_ps[:, 2:3],
                                               rstd_c[:, 0:1], cmv[:],
                                               mul_op, sub_op)
                nc.scalar.activation(Et_sb[:], tsc_ps, Exp, scale=RS)
                nc.scalar.activation(pvr_t[:, 0:1], qkvt_v[:], Copy)
                nc.tensor.matmul(pvt_ps, Et_sb[:], pvr_t[:], start=True, stop=True)
                rd1 = work.tile([S, 1], fp32, tag="rd1")
                nc.vector.reciprocal(rd1[:], pvt_ps[:, 1:2])
                nc.vector.scalar_tensor_tensor(up_col[:], pvt_ps[:, 0:1],
                                               rd1[:, 0:1], u_col[:],
                                               mul_op, add_op)
                nc.scalar.activation(up_bf[:], up_col[:], Copy)
                ts(Ub_bf[:], ones_mat_bf[:], up_col[:, 0:1], mul_op)

                # ---- neuron k,v columns (PE; stationary-load not the cost) ----
                for tl in range(TL):
                    base = 2048 + tl * 256
                    nc.tensor.matmul(kv_ps[:, 2 * tl:2 * tl + 1],
                                     tqkv[:, base:base + S],
                                     up_bf[:], start=True, stop=True)
                    nc.tensor.matmul(kv_ps[:, 2 * tl + 1:2 * tl + 2],
                                     tqkv[:, base + S:base + 2 * S],
                                     up_bf[:], start=True, stop=True)
                nc.vector.tensor_add(kvn[:], kv_ps, kvbias)
                k2 = kvn[:].rearrange("p (t k) -> p t k", k=2)
                ts(krsA[:], k2[:, 0:8, 0], RS, mul_op)
                ts(krsB[:], k2[:, 8:16, 0], RS, mul_op)
                p2 = pvr_n[:].rearrange("p (t k) -> p t k", k=2)
                nc.vector.tensor_mul(p2[:, :, 0], k2[:, :, 1], mt)
                nc.scalar.activation(p2[:, :, 1], mt, Copy)

                # ---- Q broadcast (+ q bias) in PSUM, bf16; fused-ACT half
                # (tls 8-15, scoresB) first so its exps start earliest ----
                for half, dst in ((1, scoresB), (0, scoresA)):
                    for hb in range(2):
                        ds = slice(hb * 512, (hb + 1) * 512)
                        cs = slice(half * 1024 + hb * 512,
                                   half * 1024 + (hb + 1) * 512)
                        nc.tensor.matmul(dst[:, ds], Ub_bf[:], tqkv[:, cs],
                                         start=True, stop=False,
                                         skip_group_check=True)
                        nc.tensor.matmul(dst[:, ds], ones_row_bf[:],
                                         bqr[:, cs], start=False, stop=True,
                                         skip_group_check=True)

                # ---- k*rs scale + exp: fused-ACT for tls 8-15 (tiles B),
                # DVE-scale + two wide ACT exps for tls 0-7 (tiles A).
                # Disjoint tiles per half so the scheduler can't tie the
                # engines together with tile-granular edges. ----
                for j in range(8):
                    tl = 8 + j
                    nc.scalar.activation(E_B[:, j * S:(j + 1) * S],
                                         scoresB[:, j * S:(j + 1) * S],
                                         Exp, scale=krsB[:, j:j + 1])
                    nc.tensor.matmul(pvn_ps[:, 2 * tl:2 * tl + 2],
                                     E_B[:, j * S:(j + 1) * S],
                                     pvr_n[:, 2 * tl:2 * tl + 2],
                                     start=True, stop=True)
                for half in range(2):
                    for j in range(4):
                        tl = 4 * half + j
                        ts(sc_sb[:, tl * S:(tl + 1) * S],
                           scoresA[:, tl * S:(tl + 1) * S],
                           krsA[:, tl:tl + 1], mul_op)
                    gs = slice(half * 512, (half + 1) * 512)
                    nc.scalar.activation(E_A[:, gs], sc_sb[:, gs], Exp)
                    for j in range(4):
                        tl = 4 * half + j
                        nc.tensor.matmul(pvn_ps[:, 2 * tl:2 * tl + 2],
                                         E_A[:, tl * S:(tl + 1) * S],
                                         pvr_n[:, 2 * tl:2 * tl + 2],
                                         start=True, stop=True)

                pv2 = pvn_ps.rearrange("p (t k) -> p t k", k=2)
                nc.vector.reciprocal(rden[:], pv2[:, :, 1])
                nc.vector.tensor_mul(zp[:], pv2[:, :, 0], rden[:])

                # ---- aff = sum_i wmt*(zp + u') + wbias  (fp32) ----
                nc.vector.tensor_mul(afr[:, 0:TL], wmt, zp[:])
                ts(afr[:, TL:2 * TL], wmt, up_col[:, 0:1], mul_op)
                nc.tensor.matmul(af_ps, afr[:, 0:TL], ones_col[:],
                                 start=True, stop=False, skip_group_check=True)
                nc.tensor.matmul(af_ps, afr[:, TL:2 * TL], ones_col[:],
                                 start=False, stop=False, skip_group_check=True)
                nc.tensor.matmul(af_ps, wbr[:, b * TL:(b + 1) * TL], one_one[:],
                                 start=False, stop=True, skip_group_check=True)

                if b == L - 1:
                    nc.vector.tensor_copy(aff_sb[:], af_ps)
                    nc.sync.dma_start(out_d, aff_sb[:])
                else:
                    nc.vector.tensor_copy(aff_sb[:], af_ps)
                    nc.tensor.matmul(v128_ps, sel[:], aff_sb[:],
                                     start=True, stop=True)
                    # adaptive gelu (tanh approx), sel-placed column
                    g0 = pre[:, 2 + b:3 + b]
                    g1h = pre[:, 10 + b:11 + b]
                    xg = work.tile([S, 1], fp32, tag="xg")
                    s2t = work.tile([S, 1], fp32, tag="s2t")
                    t1 = work.tile([S, 1], fp32, tag="t1")
                    ts(xg[:], v128_ps, g0[:, 0:1], mul_op)
                    nc.vector.tensor_mul(s2t[:], xg[:], xg[:])
                    ts(t1[:], s2t[:], GA, mul_op, 1.0, add_op)
                    nc.vector.tensor_mul(t1[:], t1[:], xg[:])
                    nc.scalar.activation(t1[:], t1[:], Tanh, scale=GC)
                    nc.vector.scalar_tensor_tensor(t1[:], t1[:], 1.0, xg[:],
                                                   add_op, mul_op)
                    # WAW anchor for the next batch's reduce: without it the
                    # scheduler hoists the (sem-blocked) reduce to the head
                    # of the in-order DVE queue and wedges the whole engine.
                    # Reads t1 (not bsrc) to stay off the trigger's WAR path.
                    ts(v_tmp[0:1, 0:1], ones_row[0:1, 0:1],
                       t1[0:1, 0:1], mul_op)
                    ts(bsrc[b][:], t1[:], g1h[:, 0:1], mul_op)
                    nc.gpsimd.trigger_dma(count=None,
                                          signals_writable=[bsrc[b][:]])

    nc.compile()
    return nc


def _host_prep(x, W, mask, attn_t, attn_n, norm_params, ada):
    import ml_dtypes
    f32 = np.float32
    bf16 = ml_dtypes.bfloat16
    x, W, mask, attn_t, attn_n, norm_params, ada = (
        np.ascontiguousarray(np.asarray(a, f32))
        for a in (x, W, mask, attn_t, attn_n, norm_params, ada))
    gamma = norm_params[:, 0, :]
    beta = norm_params[:, 1, :]

    topo_w = attn_t[:, :, :, :S]
    topo_b = attn_t[:, :, :, S]
    topo_wg = topo_w * gamma[:, None, None, :]
    topo_wt_flat = np.ascontiguousarray(
        topo_wg.transpose(0, 3, 1, 2)).reshape(L, S, 3 * S)
    topo_c = topo_wg.sum(axis=3)
    topo_bp = np.einsum('lmis,ls->lmi', topo_w, beta) + topo_b

    wmat = W[:, :, :S] * mask
    wbias = W[:, :, S]

    pre = np.zeros((S, 178), f32)
    pre[:, 0] = x
    pre[:, 2:10] = ada[:, :, 0].T
    pre[:, 10:18] = (0.5 * ada[:, :, 1]).astype(f32).T
    pre[:, 146:154] = gamma.T
    pre[:, 154:162] = beta.T
    pre[:, 162:170] = topo_c[:, 2, :].T
    pre[:, 170:178] = topo_bp[:, 2, :].T

    ident = np.eye(S, dtype=f32)
    magic = np.array([[MAGIC, 0]], np.int32)
    thr = np.full((1, 8), 14, np.int32)
    
    in_maps = []
    for c in range(N_CORES):
        sl = slice(c * TL, (c + 1) * TL)
        an = attn_n[:, sl]
        anw = an[:, :, :, :, :S]                              # (L,TL,3,i,p)
        anb = an[:, :, :, :, S]                               # (L,TL,3,i)
        qpart = np.ascontiguousarray(
            anw[:, :, 0].transpose(0, 3, 1, 2)).reshape(L, S, TL * S)
        kvpart = np.ascontiguousarray(
            anw[:, :, 1:3].transpose(0, 4, 1, 2, 3)).reshape(L, S, TL * 2 * S)
        kv = np.stack([anb[:, :, 1, :], anb[:, :, 2, :]], axis=2)  # (L,TL,2,i)
        kvb = kv.transpose(0, 3, 1, 2).reshape(L, S, 2 * TL)
        mtt = mask[:, sl].transpose(0, 2, 1)
        tqkv = np.concatenate([qpart, kvpart, topo_wt_flat,
                               kvb, mtt], axis=2).astype(bf16)
        premap = pre.copy()
        premap[:, 18:146] = wmat[:, sl].transpose(0, 2, 1).transpose(
            1, 0, 2).reshape(S, L * TL)
        bqr = np.ascontiguousarray(
            anb[:, :, 0, :].reshape(L, TL * S)).astype(bf16)
        tcqk = topo_c[:, 0:2, :].reshape(L, 2 * S)
        bpqk = topo_bp[:, 0:2, :].reshape(L, 2 * S)
        trow = np.ascontiguousarray(np.concatenate(
            [tcqk, bpqk], axis=1).reshape(1, L * 4 * S)).astype(bf16)
        selm = np.zeros((TL, S), f32)
        for j in range(TL):
            selm[j, c * TL + j] = 1.0
        wbr = np.ascontiguousarray(wbias[:, sl].reshape(1, L * TL))
        in_maps.append(dict(tqkv=tqkv, trow=trow,
                            bqr=bqr, pre=premap, sel=selm, wbr=wbr,
                            thr=thr, magic=magic))
    return in_maps


def kernel(x, W, mask, attn_t, attn_n, attn_mask_n, norm_params, ada,
           span_ids, tb_ids):
    global _cached
    _patch_topology()
    from concourse import bass_utils
    if _cached is None:
        _cached = _build()
    nc = _cached
    in_maps = _host_prep(x, W, mask, attn_t, attn_n, norm_params, ada)
    res = bass_utils.run_bass_kernel_spmd(nc, in_maps, core_ids=list(range(N_CORES)))
    out = np.concatenate([res.results[c]["out"].reshape(TL) for c in range(N_CORES)])
    return out.astype(np.float32)


# revision 6
# speedup vs baseline: 1.0971x; 1.0254x over previous
"""v3 Trainium2 Bass kernel.

Scheme (per core c of 8, per topo batch b of 8):
  gather v (remote-DMA all-gather, NOT gpsimd collective) -> layernorm stats
  (Newton-1 rsqrt) -> topo self-attention on the span -> per-neuron
  self-attention for this core's TL=16 neurons (bf16 matmuls; k*rs scale
  built as a PE matmul krep = krsT @ dsel and applied as one DVE multiply
  per 512-col bank, then one ACT exp per bank) -> masked affine (fp32)
  -> adaptive gelu on the sel-placed [128,1] column -> remote_dma_broadcast
  to all 8 cores' SBUF (slot k -> tpb my^k), wait on per-batch remote sem.

v3 changes vs v2 (327.7us):
  - All heavy matmuls bf16 (1 cy/row vs 4 for fp32); affine stays fp32.
  - gpsimd AllGather (15us/call in the cost model) replaced by 8
    single-dest remote_dma_broadcast preps + one trigger per batch
    (~1-2us); per-batch remote semaphores, register-valued wait threshold
    (schedule-time sim cannot constant-fold it).
  - k*rs fused via krep matmul instead of 16 per-tl tensor_scalars.
  - Copies/casts moved to ACT; stats chain shortened; single Newton iter.
"""
import sys
import numpy as np

sys.path.insert(0, "/opt/trn_rl_repo")

I, L, T, S = 128, 8, 128, 128
N_CORES = 8
TL = T // N_CORES
EPS = 1e-5
RS = float(1.0 / np.sqrt(np.float32(S)))
GC = 0.7978845608028654
GA = 0.044715
MAGIC = 0x5F3759DF

_cached = None


def _patch_topology():
    """No /dev/neuron* client-side: give the sim the static TRN2 NC map it
    needs to route remote DMA (the NEFF itself uses relative XOR routing)."""
    from concourse import libnrt
    base = (0, 1, 2, 3, 6, 7, 4, 5)

    def get_trn2_nc_mapping():
        return {(d, k): base[k] for d in range(16) for k in range(8)}

    def nc_to_real_nc(device_index, nc_index):
        return base[nc_index]

    def pnc_id_to_device_and_real_nc_index(core_id):
        return core_id // 8, base[core_id % 8]

    def get_device_id_to_routing_id_mapping():
        return {d: d for d in range(16)}

    libnrt.get_trn2_nc_mapping = get_trn2_nc_mapping
    libnrt.nc_to_real_nc = nc_to_real_nc
    libnrt.pnc_id_to_device_and_real_nc_index = pnc_id_to_device_and_real_nc_index
    libnrt.get_device_id_to_routing_id_mapping = get_device_id_to_routing_id_mapping
    for modname in ("concourse.bass_interp", "concourse.dge_state"):
        m = sys.modules.get(modname)
        if m is None:
            continue
        for fn in (nc_to_real_nc, pnc_id_to_device_and_real_nc_index,
                   get_device_id_to_routing_id_mapping):
            if hasattr(m, fn.__name__):
                setattr(m, fn.__name__, fn)


def _build():
    _patch_topology()
    from concourse import bacc, tile, mybir

    fp32 = mybir.dt.float32
    bf16 = mybir.dt.bfloat16
    int32 = mybir.dt.int32
    Exp = mybir.ActivationFunctionType.Exp
    Tanh = mybir.ActivationFunctionType.Tanh
    Copy = mybir.ActivationFunctionType.Copy
    Ident = mybir.ActivationFunctionType.Identity
    mul_op = mybir.AluOpType.mult
    add_op = mybir.AluOpType.add
    sub_op = mybir.AluOpType.subtract
    shr_op = mybir.AluOpType.arith_shift_right
    AxX = mybir.AxisListType.X

    nc = bacc.Bacc("TRN2", target_bir_lowering=False, debug=False,
                   enable_asserts=True, num_devices=N_CORES)

    tqkv_d = nc.dram_tensor("tqkv", [L, S, 6576], bf16,
                            kind="ExternalInput").ap()  # Q | k,v | topo | kvbias,mt
    bqr_d = nc.dram_tensor("bqr", [L, TL * S], bf16, kind="ExternalInput").ap()
    trow_d = nc.dram_tensor("trow", [1, L * 4 * S], bf16, kind="ExternalInput").ap()
    pre_d = nc.dram_tensor("pre", [S, 178], fp32, kind="ExternalInput").ap()
    sel_d = nc.dram_tensor("sel", [TL, S], fp32, kind="ExternalInput").ap()
    wbr_d = nc.dram_tensor("wbr", [1, L * TL], fp32, kind="ExternalInput").ap()
    thr_d = nc.dram_tensor("thr", [1, 8], int32, kind="ExternalInput").ap()
    magic_d = nc.dram_tensor("magic", [1, 2], int32, kind="ExternalInput").ap()
    out_d = nc.dram_tensor("out", [TL, 1], fp32, kind="ExternalOutput").ap()

    rsems = [nc.alloc_semaphore(f"rsem{b}") for b in range(L - 1)]
    lsem = nc.alloc_semaphore("lsem")
    bsem = nc.alloc_semaphore("bsem")
    gsem = nc.alloc_semaphore("gsem")

    with tile.TileContext(nc) as tc:
        with tc.tile_pool(name="wpool", bufs=3) as wpool, \
             tc.tile_pool(name="spool", bufs=3) as spool, \
             tc.tile_pool(name="fixed", bufs=1) as fixed, \
             tc.tile_pool(name="work", bufs=1) as work, \
             tc.tile_pool(name="ps_big", bufs=1, space="PSUM") as ps_big, \
             tc.tile_pool(name="ps_sm", bufs=1, space="PSUM") as ps_sm, \
             tc.tile_pool(name="ps_tp", bufs=1, space="PSUM") as ps_tp:

            pre = fixed.tile([S, 178], fp32)
            nc.sync.dma_start(pre[:], pre_d)
            magic = fixed.tile([1, 2], int32)
            nc.scalar.dma_start(magic[:], magic_d)
            trow = fixed.tile([1, L * 4 * S], bf16)
            nc.scalar.dma_start(trow[:], trow_d)
            thr = fixed.tile([1, 8], int32)
            sel = fixed.tile([TL, S], fp32)
            wbr = fixed.tile([1, L * TL], fp32)
            ones_col = fixed.tile([S, 1], fp32)
            nc.vector.memset(ones_col[:], 1.0)
            ones_row = fixed.tile([1, S], fp32)
            nc.vector.memset(ones_row[:], 1.0)
            ones_row_bf = fixed.tile([1, S], bf16)
            nc.vector.memset(ones_row_bf[:], 1.0)
            ones_mat_bf = fixed.tile([S, S], bf16)
            nc.vector.memset(ones_mat_bf[:], 1.0)
            one_one = fixed.tile([1, 1], fp32)
            nc.vector.memset(one_one[:], 1.0)
            pvr_t = fixed.tile([S, 2], bf16)
            nc.vector.memset(pvr_t[:], 1.0)

            bsrc = [fixed.tile([S, 1], fp32, name=f"bsrc{b}") for b in range(L - 1)]
            v8s = [fixed.tile([S, 7], fp32, name=f"v8_{b}") for b in range(L - 1)]

            v_col = work.tile([S, 1], fp32)
            u_col = work.tile([S, 1], fp32)
            up_col = work.tile([S, 1], fp32)
            up_bf = work.tile([S, 1], bf16)
            v_bf = work.tile([S, 1], bf16)
            Ub_bf = work.tile([S, S], bf16)
            sc = work.tile([1, 12], fp32)
            sci = sc[:].bitcast(int32)
            bc_sb = work.tile([S, 2], fp32)
            qkvt_c = work.tile([S, 3], fp32)
            qk_row = work.tile([1, 2 * S], fp32)
            cm2_row = work.tile([1, 2 * S], fp32)
            cmv = work.tile([S, 1], fp32)
            qkvt_v = work.tile([S, 1], fp32)
            Et_sb = work.tile([S, S], bf16)
            v_tmp = work.tile([S, 1], fp32)
            kvn = work.tile([S, 2 * TL], fp32)
            krsA = work.tile([S, TL // 2], fp32)
            krsB = work.tile([S, TL // 2], fp32)
            pvr_n = work.tile([S, 2 * TL], bf16)
            sc_sb = work.tile([S, TL * S // 2], bf16)
            E_A = work.tile([S, TL * S // 2], bf16)
            E_B = work.tile([S, TL * S // 2], bf16)
            rden = work.tile([S, TL], fp32)
            zp = work.tile([S, TL], fp32)
            afr = work.tile([S, 2 * TL], fp32)
            aff_sb = work.tile([TL, 1], fp32)

            scoresA = ps_big.tile([S, 1024], fp32)       # 2 banks (tls 0-7)
            scoresB = ps_big.tile([S, 1024], fp32)       # 2 banks (tls 8-15)
            smps = ps_sm.tile([S, 512], fp32)            # 1 bank
            kv_ps = smps[:, 0:32]
            pvn_ps = smps[:, 32:64]
            af_ps = smps[0:TL, 64:65]
            v128_ps = smps[:, 66:67]
            sv_ps = smps[0:1, 68:69]
            svv_ps = smps[0:1, 69:70]
            bc_ps = smps[:, 70:72]
            A_ps = smps[:, 72:75]
            pvt_ps = smps[:, 76:78]
            tpps = ps_tp.tile([S, 512], fp32)            # 1 bank (topo stage)
            qk0_ps = tpps[0:1, 128:384]
            tsc_ps = tpps[:, 0:128]

            rthr_cm = nc.vector.register("rthr")
            rthr = rthr_cm.__enter__()
            nc.vector.reg_load(rthr, thr[0:1, 0:1])

            # Tracked WAW edge: reg_save writes a byte of v_col, so every
            # later v_col writer (incl. the sem-waiting reduce) orders after
            # the reg_load (register deps inside wait conditions are not
            # tracked by tile).
            nc.vector.reg_save(v_tmp[0:1, 0:1].bitcast(int32), rthr)

            def ts(out, in0, s1, op0, s2=None, op1=None, eng=None):
                e = eng or nc.vector
                if s2 is None:
                    e.tensor_scalar(out, in0, s1, None, op0)
                else:
                    e.tensor_scalar(out, in0, s1, s2, op0, op1)

            for b in range(L):
                # ---- weight prefetch (tqkv split in 4 so the gather trigger
                # never queues behind a >1us DMA) ----
                tqkv = wpool.tile([S, 6576], bf16, tag="tqkv")
                bqr = spool.tile([1, TL * S], bf16, tag="bqr")
                for q in ((2, 1, 0) if b == 0 else range(3)):
                    nc.sync.dma_start(tqkv[:, q * 2192:(q + 1) * 2192],
                                      tqkv_d[b][:, q * 2192:(q + 1) * 2192])
                if b > 0:
                    nc.sync.dma_start(bqr[:], bqr_d[b])
                if b == 0:
                    nc.sync.dma_start(thr[:], thr_d)
                    nc.sync.dma_start(bqr[:], bqr_d[b])
                    # Tracked WAW edge: reg_save writes a byte of v_tmp, so
                    # every later v_tmp writer (incl. the sem-waiting reduce)
                    # orders after the reg_load (register deps inside wait
                    # conditions are not tracked by tile).
                    nc.vector.reg_load(rthr, thr[0:1, 0:1])
                    nc.vector.reg_save(v_tmp[0:1, 0:1].bitcast(int32), rthr)
                    # Tracked WAW edge: reg_save writes a byte of v_tmp, so
                    # every later v_tmp writer (incl. the sem-waiting reduce)
                    # orders after the reg_load (register deps inside wait
                    # conditions are not tracked by tile).
                    nc.vector.reg_load(rthr, thr[0:1, 0:1])
                    nc.vector.reg_save(v_tmp[0:1, 0:1].bitcast(int32), rthr)
                if b == 0:
                    # deferred: keeps the early ACT queue clear for batch 0;
                    # these land ~12us, first use ~15us (batch-0 tail)
                    nc.scalar.dma_start(sel[:], sel_d)
                    nc.scalar.dma_start(wbr[:], wbr_d)
                kvbias = tqkv[:, 6528:6560]
                mt = tqkv[:, 6560:6576]
                wmt = pre[:, 18 + 16 * b:18 + 16 * (b + 1)]
                gam = pre[:, 146 + b:147 + b]
                bet = pre[:, 154 + b:155 + b]

                # ---- desc-gen for THIS batch's end-of-batch broadcast (the
                # trigger at the end of this batch fires these 8 preps) ----
                if b < L - 1:
                    if b >= 2:
                        # SWDGE ring holds ~14 preps. Dummy write to bsrc[b]
                        # reading bsrc[b-2] (a declared output of trigger
                        # b-2): the preps' no-sync src edge then orders them
                        # after trigger b-2 on the in-order Pool queue, so
                        # ring entries are reclaimed before desc-gen.
                        ts(bsrc[b][0:1, 0:1], ones_row[0:1, 0:1],
                           bsrc[b - 2][0:1, 0:1], mul_op)
                    for k in range(1, N_CORES):
                        rd = [None] * 8
                        rd[k] = (0, k)
                        nc.gpsimd.remote_dma_broadcast(
                            v8s[b][:, k - 1:k], bsrc[b][:],
                            rsems[b], lsem, rdests=rd)

                # ---- acquire v ----
                if b == 0:
                    nc.vector.tensor_copy(v_col[:], pre[:, 0:1])
                else:
                    red = nc.vector.tensor_reduce(v_tmp[:], v8s[b - 1][:],
                                                  AxX, add_op)
                    red.wait_op(rsems[b - 1], rthr, "sem-ge")
                    nc.vector.tensor_add(v_col[:], v_tmp[:], bsrc[b - 1][:])

                # ---- topo qkv on raw v (PE, parallel with stats) ----
                nc.vector.tensor_copy(v_bf[:], v_col[:])
                nc.tensor.matmul(qk0_ps, v_bf[:], tqkv[:, 6144:6144 + 2 * S],
                                 start=True, stop=True)
                nc.tensor.matmul(A_ps[:, 2:3],
                                 tqkv[:, 6144 + 2 * S:6144 + 3 * S],
                                 v_bf[:], start=True, stop=True)

                # ---- stats + Newton-1 rsqrt ----
                nc.tensor.matmul(sv_ps, ones_col[:], v_col[:], start=True, stop=True)
                nc.tensor.matmul(svv_ps, v_col[:], v_col[:], start=True, stop=True)
                ts(sc[:, 0:1], sv_ps, 1.0 / S, mul_op)
                ts(sc[:, 1:2], svv_ps, 1.0 / S, mul_op)
                nc.vector.scalar_tensor_tensor(sc[:, 3:4], sc[:, 0:1], sc[:, 0:1],
                                               sc[:, 1:2], mul_op, sub_op)
                ts(sc[:, 4:5], sc[:, 3:4], -1.0, mul_op, EPS, add_op)      # vpe
                ts(sc[:, 5:6], sc[:, 3:4], -0.5, mul_op, 0.5 * EPS, add_op)  # vh
                ts(sci[:, 8:9], sci[:, 4:5], 1, shr_op)
                nc.vector.tensor_sub(sci[:, 6:7], magic[:, 0:1], sci[:, 8:9])
                nc.vector.scalar_tensor_tensor(sc[:, 8:9], sc[:, 6:7], sc[:, 5:6],
                                               sc[:, 6:7], mul_op, mul_op)
                ts(sc[:, 8:9], sc[:, 8:9], -1.0, mul_op, 1.5, add_op)
                nc.vector.tensor_mul(sc[:, 6:7], sc[:, 6:7], sc[:, 8:9])   # rstd
                nc.vector.tensor_mul(sc[:, 7:8], sc[:, 6:7], sc[:, 0:1])   # mu*rstd
                nc.tensor.matmul(bc_ps, ones_row[:], sc[:, 6:8], start=True, stop=True)
                nc.scalar.activation(bc_sb[:], bc_ps, Copy)
                rstd_c = bc_sb[:, 0:1]
                murstd_c = bc_sb[:, 1:2]

                # ---- u = rstd*gamma*(v-mu) + beta  (ACT) ----
                grstd = work.tile([S, 1], fp32, tag="grstd")
                gmr = work.tile([S, 1], fp32, tag="gmr")
                boff = work.tile([S, 1], fp32, tag="boff")
                ts(grstd[:], gam, rstd_c, mul_op)
                ts(gmr[:], gam, murstd_c, mul_op)
                nc.vector.tensor_sub(boff[:], bet, gmr[:])
                nc.scalar.activation(u_col[:], v_col[:], Ident,
                                     bias=boff[:, 0:1], scale=grstd[:, 0:1])

                # ---- topo attention: q,k corrected in row space (the
                # scalars live at partition 0 in sc, no broadcast needed) ----
                tc_row = trow[0:1, b * 512:b * 512 + 2 * S]
                bp_row = trow[0:1, b * 512 + 2 * S:(b + 1) * 512]
                nc.vector.scalar_tensor_tensor(cm2_row[:], tc_row, sc[:, 7:8],
                                               bp_row, mul_op, sub_op)
                nc.vector.scalar_tensor_tensor(qk_row[:], qk0_ps, sc[:, 6:7],
                                               cm2_row[:], mul_op, sub_op)
                nc.tensor.matmul(tsc_ps, qk_row[0:1, S:2 * S],
                                 qk_row[0:1, 0:S], start=True, stop=True)
                nc.vector.scalar_tensor_tensor(cmv[:], pre[:, 162 + b:163 + b],
                                               murstd_c[:, 0:1],
                                               pre[:, 170 + b:171 + b],
                                               mul_op, sub_op)
                nc.vector.scalar_tensor_tensor(qkvt_v[:], A_ps[:, 2:3],
                                               rstd_c[:, 0:1], cmv[:],
                                               mul_op, sub_op)
                nc.scalar.activation(Et_sb[:], tsc_ps, Exp, scale=RS)
                nc.scalar.activation(pvr_t[:, 0:1], qkvt_v[:], Copy)
                nc.tensor.matmul(pvt_ps, Et_sb[:], pvr_t[:], start=True, stop=True)
                rd1 = work.tile([S, 1], fp32, tag="rd1")
                nc.vector.reciprocal(rd1[:], pvt_ps[:, 1:2])
                nc.vector.scalar_tensor_tensor(up_col[:], pvt_ps[:, 0:1],
                                               rd1[:, 0:1], u_col[:],
                                               mul_op, add_op)
                nc.scalar.activation(up_bf[:], up_col[:], Copy)
                ts(Ub_bf[:], ones_mat_bf[:], up_col[:, 0:1], mul_op)

                # ---- neuron k,v columns (PE; stationary-load not the cost) ----
                for tl in range(TL):
                    base = 2048 + tl * 256
                    nc.tensor.matmul(kv_ps[:, 2 * tl:2 * tl + 1],
                                     tqkv[:, base:base + S],
                                     up_bf[:], start=True, stop=True)
                    nc.tensor.matmul(kv_ps[:, 2 * tl + 1:2 * tl + 2],
                                     tqkv[:, base + S:base + 2 * S],
                                     up_bf[:], start=True, stop=True)
                nc.vector.tensor_add(kvn[:], kv_ps, kvbias)
                k2 = kvn[:].rearrange("p (t k) -> p t k", k=2)
                ts(krsA[:], k2[:, 0:8, 0], RS, mul_op)
                ts(krsB[:], k2[:, 8:16, 0], RS, mul_op)
                p2 = pvr_n[:].rearrange("p (t k) -> p t k", k=2)
                nc.vector.tensor_mul(p2[:, :, 0], k2[:, :, 1], mt)
                nc.scalar.activation(p2[:, :, 1], mt, Copy)

                # ---- Q broadcast (+ q bias) in PSUM, bf16; fused-ACT half
                # (tls 8-15, scoresB) first so its exps start earliest ----
                for half, dst in ((1, scoresB), (0, scoresA)):
                    for hb in range(2):
                        ds = slice(hb * 512, (hb + 1) * 512)
                        cs = slice(half * 1024 + hb * 512,
                                   half * 1024 + (hb + 1) * 512)
                        nc.tensor.matmul(dst[:, ds], Ub_bf[:], tqkv[:, cs],
                                         start=True, stop=False,
                                         skip_group_check=True)
                        nc.tensor.matmul(dst[:, ds], ones_row_bf[:],
                                         bqr[:, cs], start=False, stop=True,
                                         skip_group_check=True)

                # ---- k*rs scale + exp: fused-ACT for tls 8-15 (tiles B),
                # DVE-scale + two wide ACT exps for tls 0-7 (tiles A).
                # Disjoint tiles per half so the scheduler can't tie the
                # engines together with tile-granular edges. ----
                for j in range(8):
                    tl = 8 + j
                    nc.scalar.activation(E_B[:, j * S:(j + 1) * S],
                                         scoresB[:, j * S:(j + 1) * S],
                                         Exp, scale=krsB[:, j:j + 1])
                    nc.tensor.matmul(pvn_ps[:, 2 * tl:2 * tl + 2],
                                     E_B[:, j * S:(j + 1) * S],
                                     pvr_n[:, 2 * tl:2 * tl + 2],
                                     start=True, stop=True)
                for half in range(2):
                    for j in range(4):
                        tl = 4 * half + j
                        ts(sc_sb[:, tl * S:(tl + 1) * S],
                           scoresA[:, tl * S:(tl + 1) * S],
                           krsA[:, tl:tl + 1], mul_op)
                    gs = slice(half * 512, (half + 1) * 512)
                    nc.scalar.activation(E_A[:, gs], sc_sb[:, gs], Exp)
                    for j in range(4):
                        tl = 4 * half + j
                        nc.tensor.matmul(pvn_ps[:, 2 * tl:2 * tl + 2],
                                         E_A[:, tl * S:(tl + 1) * S],
                                         pvr_n[:, 2 * tl:2 * tl + 2],
                                         start=True, stop=True)

                pv2 = pvn_ps.rearrange("p (t k) -> p t k", k=2)
                nc.vector.reciprocal(rden[:], pv2[:, :, 1])
                nc.vector.tensor_mul(zp[:], pv2[:, :, 0], rden[:])

                # ---- aff = sum_i wmt*(zp + u') + wbias  (fp32) ----
                nc.vector.tensor_mul(afr[:, 0:TL], wmt, zp[:])
                ts(afr[:, TL:2 * TL], wmt, up_col[:, 0:1], mul_op)
                nc.tensor.matmul(af_ps, afr[:, 0:TL], ones_col[:],
                                 start=True, stop=False, skip_group_check=True)
                nc.tensor.matmul(af_ps, afr[:, TL:2 * TL], ones_col[:],
                                 start=False, stop=False, skip_group_check=True)
                nc.tensor.matmul(af_ps, wbr[:, b * TL:(b + 1) * TL], one_one[:],
                                 start=False, stop=True, skip_group_check=True)

                if b == L - 1:
                    nc.vector.tensor_copy(aff_sb[:], af_ps)
                    nc.sync.dma_start(out_d, aff_sb[:])
                else:
                    nc.vector.tensor_copy(aff_sb[:], af_ps)
                    nc.tensor.matmul(v128_ps, sel[:], aff_sb[:],
                                     start=True, stop=True)
                    # adaptive gelu (tanh approx), sel-placed column
                    g0 = pre[:, 2 + b:3 + b]
                    g1h = pre[:, 10 + b:11 + b]
                    xg = work.tile([S, 1], fp32, tag="xg")
                    s2t = work.tile([S, 1], fp32, tag="s2t")
                    t1 = work.tile([S, 1], fp32, tag="t1")
                    ts(xg[:], v128_ps, g0[:, 0:1], mul_op)
                    nc.vector.tensor_mul(s2t[:], xg[:], xg[:])
                    ts(t1[:], s2t[:], GA, mul_op, 1.0, add_op)
                    nc.vector.tensor_mul(t1[:], t1[:], xg[:])
                    nc.scalar.activation(t1[:], t1[:], Tanh, scale=GC)
                    nc.vector.scalar_tensor_tensor(t1[:], t1[:], 1.0, xg[:],
                                                   add_op, mul_op)
                    # WAW anchor for the next batch's reduce: without it the
                    # scheduler hoists the (sem-blocked) reduce to the head
                    # of the in-order DVE queue and wedges the whole engine.
                    # Reads t1 (not bsrc) to stay off the trigger's WAR path.
                    ts(v_tmp[0:1, 0:1], ones_row[0:1, 0:1],
                       t1[0:1, 0:1], mul_op)
                    ts(bsrc[b][:], t1[:], g1h[:, 0:1], mul_op)
                    nc.gpsimd.trigger_dma(count=None,
                                          signals_writable=[bsrc[b][:]])

    nc.compile()
    return nc


def _host_prep(x, W, mask, attn_t, attn_n, norm_params, ada):
    import ml_dtypes
    f32 = np.float32
    bf16 = ml_dtypes.bfloat16
    x, W, mask, attn_t, attn_n, norm_params, ada = (
        np.ascontiguousarray(np.asarray(a, f32))
        for a in (x, W, mask, attn_t, attn_n, norm_params, ada))
    gamma = norm_params[:, 0, :]
    beta = norm_params[:, 1, :]

    topo_w = attn_t[:, :, :, :S]
    topo_b = attn_t[:, :, :, S]
    topo_wg = topo_w * gamma[:, None, None, :]
    topo_wt_flat = np.ascontiguousarray(
        topo_wg.transpose(0, 3, 1, 2)).reshape(L, S, 3 * S)
    topo_c = topo_wg.sum(axis=3)
    topo_bp = np.einsum('lmis,ls->lmi', topo_w, beta) + topo_b

    wmat = W[:, :, :S] * mask
    wbias = W[:, :, S]

    pre = np.zeros((S, 178), f32)
    pre[:, 0] = x
    pre[:, 2:10] = ada[:, :, 0].T
    pre[:, 10:18] = (0.5 * ada[:, :, 1]).astype(f32).T
    pre[:, 146:154] = gamma.T
    pre[:, 154:162] = beta.T
    pre[:, 162:170] = topo_c[:, 2, :].T
    pre[:, 170:178] = topo_bp[:, 2, :].T

    ident = np.eye(S, dtype=f32)
    magic = np.array([[MAGIC, 0]], np.int32)
    thr = np.full((1, 8), 14, np.int32)
    
    in_maps = []
    for c in range(N_CORES):
        sl = slice(c * TL, (c + 1) * TL)
        an = attn_n[:, sl]
        anw = an[:, :, :, :, :S]                              # (L,TL,3,i,p)
        anb = an[:, :, :, :, S]                               # (L,TL,3,i)
        qpart = np.ascontiguousarray(
            anw[:, :, 0].transpose(0, 3, 1, 2)).reshape(L, S, TL * S)
        kvpart = np.ascontiguousarray(
            anw[:, :, 1:3].transpose(0, 4, 1, 2, 3)).reshape(L, S, TL * 2 * S)
        kv = np.stack([anb[:, :, 1, :], anb[:, :, 2, :]], axis=2)  # (L,TL,2,i)
        kvb = kv.transpose(0, 3, 1, 2).reshape(L, S, 2 * TL)
        mtt = mask[:, sl].transpose(0, 2, 1)
        tqkv = np.concatenate([qpart, kvpart, topo_wt_flat,
                               kvb, mtt], axis=2).astype(bf16)
        premap = pre.copy()
        premap[:, 18:146] = wmat[:, sl].transpose(0, 2, 1).transpose(
            1, 0, 2).reshape(S, L * TL)
        bqr = np.ascontiguousarray(
            anb[:, :, 0, :].reshape(L, TL * S)).astype(bf16)
        tcqk = topo_c[:, 0:2, :].reshape(L, 2 * S)
        bpqk = topo_bp[:, 0:2, :].reshape(L, 2 * S)
        trow = np.ascontiguousarray(np.concatenate(
            [tcqk, bpqk], axis=1).reshape(1, L * 4 * S)).astype(bf16)
        selm = np.zeros((TL, S), f32)
        for j in range(TL):
            selm[j, c * TL + j] = 1.0
        wbr = np.ascontiguousarray(wbias[:, sl].reshape(1, L * TL))
        in_maps.append(dict(tqkv=tqkv, trow=trow,
                            bqr=bqr, pre=premap, sel=selm, wbr=wbr,
                            thr=thr, magic=magic))
    return in_maps


def kernel(x, W, mask, attn_t, attn_n, attn_mask_n, norm_params, ada,
           span_ids, tb_ids):
    global _cached
    _patch_topology()
    from concourse import bass_utils
    if _cached is None:
        _cached = _build()
    nc = _cached
    in_maps = _host_prep(x, W, mask, attn_t, attn_n, norm_params, ada)
    res = bass_utils.run_bass_kernel_spmd(nc, in_maps, core_ids=list(range(N_CORES)))
    out = np.concatenate([res.results[c]["out"].reshape(TL) for c in range(N_CORES)])
    return out.astype(np.float32)


# revision 7
# speedup vs baseline: 1.1036x; 1.0059x over previous
"""v3 Trainium2 Bass kernel.

Scheme (per core c of 8, per topo batch b of 8):
  gather v (remote-DMA all-gather, NOT gpsimd collective) -> layernorm stats
  (Newton-1 rsqrt) -> topo self-attention on the span -> per-neuron
  self-attention for this core's TL=16 neurons (bf16 matmuls; k*rs scale
  built as a PE matmul krep = krsT @ dsel and applied as one DVE multiply
  per 512-col bank, then one ACT exp per bank) -> masked affine (fp32)
  -> adaptive gelu on the sel-placed [128,1] column -> remote_dma_broadcast
  to all 8 cores' SBUF (slot k -> tpb my^k), wait on per-batch remote sem.

v3 changes vs v2 (327.7us):
  - All heavy matmuls bf16 (1 cy/row vs 4 for fp32); affine stays fp32.
  - gpsimd AllGather (15us/call in the cost model) replaced by 8
    single-dest remote_dma_broadcast preps + one trigger per batch
    (~1-2us); per-batch remote semaphores, register-valued wait threshold
    (schedule-time sim cannot constant-fold it).
  - k*rs fused via krep matmul instead of 16 per-tl tensor_scalars.
  - Copies/casts moved to ACT; stats chain shortened; single Newton iter.
"""
import sys
import numpy as np

sys.path.insert(0, "/opt/trn_rl_repo")

I, L, T, S = 128, 8, 128, 128
N_CORES = 8
TL = T // N_CORES
EPS = 1e-5
RS = float(1.0 / np.sqrt(np.float32(S)))
GC = 0.7978845608028654
GA = 0.044715
MAGIC = 0x5F3759DF

_cached = None


def _patch_topology():
    """No /dev/neuron* client-side: give the sim the static TRN2 NC map it
    needs to route remote DMA (the NEFF itself uses relative XOR routing)."""
    from concourse import libnrt
    base = (0, 1, 2, 3, 6, 7, 4, 5)

    def get_trn2_nc_mapping():
        return {(d, k): base[k] for d in range(16) for k in range(8)}

    def nc_to_real_nc(device_index, nc_index):
        return base[nc_index]

    def pnc_id_to_device_and_real_nc_index(core_id):
        return core_id // 8, base[core_id % 8]

    def get_device_id_to_routing_id_mapping():
        return {d: d for d in range(16)}

    libnrt.get_trn2_nc_mapping = get_trn2_nc_mapping
    libnrt.nc_to_real_nc = nc_to_real_nc
    libnrt.pnc_id_to_device_and_real_nc_index = pnc_id_to_device_and_real_nc_index
    libnrt.get_device_id_to_routing_id_mapping = get_device_id_to_routing_id_mapping
    for modname in ("concourse.bass_interp", "concourse.dge_state"):
        m = sys.modules.get(modname)
        if m is None:
            continue
        for fn in (nc_to_real_nc, pnc_id_to_device_and_real_nc_index,
                   get_device_id_to_routing_id_mapping):
            if hasattr(m, fn.__name__):
                setattr(m, fn.__name__, fn)


def _build():
    _patch_topology()
    from concourse import bacc, tile, mybir

    fp32 = mybir.dt.float32
    bf16 = mybir.dt.bfloat16
    int32 = mybir.dt.int32
    Exp = mybir.ActivationFunctionType.Exp
    Tanh = mybir.ActivationFunctionType.Tanh
    Copy = mybir.ActivationFunctionType.Copy
    Ident = mybir.ActivationFunctionType.Identity
    mul_op = mybir.AluOpType.mult
    add_op = mybir.AluOpType.add
    sub_op = mybir.AluOpType.subtract
    shr_op = mybir.AluOpType.arith_shift_right
    AxX = mybir.AxisListType.X

    nc = bacc.Bacc("TRN2", target_bir_lowering=False, debug=False,
                   enable_asserts=True, num_devices=N_CORES)

    tqkv_d = nc.dram_tensor("tqkv", [L, S, 6576], bf16,
                            kind="ExternalInput").ap()  # Q | k,v | topo | kvbias,mt
    bqr_d = nc.dram_tensor("bqr", [L, TL * S], bf16, kind="ExternalInput").ap()
    trow_d = nc.dram_tensor("trow", [1, L * 4 * S], bf16, kind="ExternalInput").ap()
    pre_d = nc.dram_tensor("pre", [S, 178], fp32, kind="ExternalInput").ap()
    sel_d = nc.dram_tensor("sel", [TL, S], fp32, kind="ExternalInput").ap()
    wbr_d = nc.dram_tensor("wbr", [1, L * TL], fp32, kind="ExternalInput").ap()
    thr_d = nc.dram_tensor("thr", [1, 8], int32, kind="ExternalInput").ap()
    magic_d = nc.dram_tensor("magic", [1, 2], int32, kind="ExternalInput").ap()
    out_d = nc.dram_tensor("out", [TL, 1], fp32, kind="ExternalOutput").ap()

    rsems = [nc.alloc_semaphore(f"rsem{b}") for b in range(L - 1)]
    lsem = nc.alloc_semaphore("lsem")
    bsem = nc.alloc_semaphore("bsem")
    gsem = nc.alloc_semaphore("gsem")

    with tile.TileContext(nc) as tc:
        with tc.tile_pool(name="wpool", bufs=3) as wpool, \
             tc.tile_pool(name="spool", bufs=3) as spool, \
             tc.tile_pool(name="fixed", bufs=1) as fixed, \
             tc.tile_pool(name="work", bufs=1) as work, \
             tc.tile_pool(name="ps_big", bufs=1, space="PSUM") as ps_big, \
             tc.tile_pool(name="ps_sm", bufs=1, space="PSUM") as ps_sm, \
             tc.tile_pool(name="ps_tp", bufs=1, space="PSUM") as ps_tp:

            pre = fixed.tile([S, 178], fp32)
            nc.sync.dma_start(pre[:], pre_d)
            magic = fixed.tile([1, 2], int32)
            nc.scalar.dma_start(magic[:], magic_d)
            trow = fixed.tile([1, L * 4 * S], bf16)
            nc.scalar.dma_start(trow[:], trow_d)
            thr = fixed.tile([1, 8], int32)
            sel = fixed.tile([TL, S], fp32)
            wbr = fixed.tile([1, L * TL], fp32)
            ones_col = fixed.tile([S, 1], fp32)
            nc.vector.memset(ones_col[:], 1.0)
            ones_row = fixed.tile([1, S], fp32)
            nc.vector.memset(ones_row[:], 1.0)
            ones_row_bf = fixed.tile([1, S], bf16)
            nc.vector.memset(ones_row_bf[:], 1.0)
            ones_mat_bf = fixed.tile([S, S], bf16)
            nc.vector.memset(ones_mat_bf[:], 1.0)
            one_one = fixed.tile([1, 1], fp32)
            nc.vector.memset(one_one[:], 1.0)
            pvr_t = fixed.tile([S, 2], bf16)
            nc.vector.memset(pvr_t[:], 1.0)

            bsrc = [fixed.tile([S, 1], fp32, name=f"bsrc{b}") for b in range(L - 1)]
            v8s = [fixed.tile([S, 7], fp32, name=f"v8_{b}") for b in range(L - 1)]

            v_col = work.tile([S, 1], fp32)
            u_col = work.tile([S, 1], fp32)
            up_col = work.tile([S, 1], fp32)
            up_bf = work.tile([S, 1], bf16)
            v_bf = work.tile([S, 1], bf16)
            Ub_bf = work.tile([S, S], bf16)
            sc = work.tile([1, 12], fp32)
            sci = sc[:].bitcast(int32)
            bc_sb = work.tile([S, 2], fp32)
            qkvt_c = work.tile([S, 3], fp32)
            qk_row = work.tile([1, 2 * S], fp32)
            cm2_row = work.tile([1, 2 * S], fp32)
            cmv = work.tile([S, 1], fp32)
            qkvt_v = work.tile([S, 1], fp32)
            Et_sb = work.tile([S, S], bf16)
            v_tmp = work.tile([S, 1], fp32)
            kvn = work.tile([S, 2 * TL], fp32)
            krsA = work.tile([S, TL // 2], fp32)
            krsB = work.tile([S, TL // 2], fp32)
            pvr_n = work.tile([S, 2 * TL], bf16)
            sc_sb = work.tile([S, TL * S // 2], bf16)
            E_A = work.tile([S, TL * S // 2], bf16)
            E_B = work.tile([S, TL * S // 2], bf16)
            rden = work.tile([S, TL], fp32)
            zp = work.tile([S, TL], fp32)
            afr = work.tile([S, 2 * TL], fp32)
            aff_sb = work.tile([TL, 1], fp32)

            scoresA = ps_big.tile([S, 1024], fp32)       # 2 banks (tls 0-7)
            scoresB = ps_big.tile([S, 1024], fp32)       # 2 banks (tls 8-15)
            smps = ps_sm.tile([S, 512], fp32)            # 1 bank
            kv_ps = smps[:, 0:32]
            pvn_ps = smps[:, 32:64]
            af_ps = smps[0:TL, 64:65]
            v128_ps = smps[:, 66:67]
            sv_ps = smps[0:1, 68:69]
            svv_ps = smps[0:1, 69:70]
            bc_ps = smps[:, 70:72]
            A_ps = smps[:, 72:75]
            pvt_ps = smps[:, 76:78]
            tpps = ps_tp.tile([S, 512], fp32)            # 1 bank (topo stage)
            qk0_ps = tpps[0:1, 128:384]
            tsc_ps = tpps[:, 0:128]

            rthr_cm = nc.vector.register("rthr")
            rthr = rthr_cm.__enter__()
            nc.vector.reg_load(rthr, thr[0:1, 0:1])

            # Tracked WAW edge: reg_save writes a byte of v_col, so every
            # later v_col writer (incl. the sem-waiting reduce) orders after
            # the reg_load (register deps inside wait conditions are not
            # tracked by tile).
            nc.vector.reg_save(v_tmp[0:1, 0:1].bitcast(int32), rthr)

            def ts(out, in0, s1, op0, s2=None, op1=None, eng=None):
                e = eng or nc.vector
                if s2 is None:
                    e.tensor_scalar(out, in0, s1, None, op0)
                else:
                    e.tensor_scalar(out, in0, s1, s2, op0, op1)

            for b in range(L):
                # ---- weight prefetch (tqkv split in 4 so the gather trigger
                # never queues behind a >1us DMA) ----
                tqkv = wpool.tile([S, 6576], bf16, tag="tqkv")
                bqr = spool.tile([1, TL * S], bf16, tag="bqr")
                for q in ((2, 0) if b == 0 else range(3)):
                    nc.sync.dma_start(tqkv[:, q * 2192:(q + 1) * 2192],
                                      tqkv_d[b][:, q * 2192:(q + 1) * 2192])
                if b == 0:
                    nc.scalar.dma_start(tqkv[:, 2192:2 * 2192],
                                        tqkv_d[b][:, 2192:2 * 2192])
                if b > 0:
                    nc.sync.dma_start(bqr[:], bqr_d[b])
                if b == 0:
                    nc.sync.dma_start(thr[:], thr_d)
                    nc.sync.dma_start(bqr[:], bqr_d[b])
                    # Tracked WAW edge: reg_save writes a byte of v_tmp, so
                    # every later v_tmp writer (incl. the sem-waiting reduce)
                    # orders after the reg_load (register deps inside wait
                    # conditions are not tracked by tile).
                    nc.vector.reg_load(rthr, thr[0:1, 0:1])
                    nc.vector.reg_save(v_tmp[0:1, 0:1].bitcast(int32), rthr)
                    # Tracked WAW edge: reg_save writes a byte of v_tmp, so
                    # every later v_tmp writer (incl. the sem-waiting reduce)
                    # orders after the reg_load (register deps inside wait
                    # conditions are not tracked by tile).
                    nc.vector.reg_load(rthr, thr[0:1, 0:1])
                    nc.vector.reg_save(v_tmp[0:1, 0:1].bitcast(int32), rthr)
                if b == 0:
                    # deferred: keeps the early ACT queue clear for batch 0;
                    # these land ~12us, first use ~15us (batch-0 tail)
                    nc.scalar.dma_start(sel[:], sel_d)
                    nc.scalar.dma_start(wbr[:], wbr_d)
                kvbias = tqkv[:, 6528:6560]
                mt = tqkv[:, 6560:6576]
                wmt = pre[:, 18 + 16 * b:18 + 16 * (b + 1)]
                gam = pre[:, 146 + b:147 + b]
                bet = pre[:, 154 + b:155 + b]

                # ---- desc-gen for THIS batch's end-of-batch broadcast (the
                # trigger at the end of this batch fires these 8 preps) ----
                if b < L - 1:
                    if b >= 2:
                        # SWDGE ring holds ~14 preps. Dummy write to bsrc[b]
                        # reading bsrc[b-2] (a declared output of trigger
                        # b-2): the preps' no-sync src edge then orders them
                        # after trigger b-2 on the in-order Pool queue, so
                        # ring entries are reclaimed before desc-gen.
                        ts(bsrc[b][0:1, 0:1], ones_row[0:1, 0:1],
                           bsrc[b - 2][0:1, 0:1], mul_op)
                    for k in range(1, N_CORES):
                        rd = [None] * 8
                        rd[k] = (0, k)
                        nc.gpsimd.remote_dma_broadcast(
                            v8s[b][:, k - 1:k], bsrc[b][:],
                            rsems[b], lsem, rdests=rd)

                # ---- acquire v ----
                if b == 0:
                    nc.vector.tensor_copy(v_col[:], pre[:, 0:1])
                else:
                    red = nc.vector.tensor_reduce(v_tmp[:], v8s[b - 1][:],
                                                  AxX, add_op)
                    red.wait_op(rsems[b - 1], rthr, "sem-ge")
                    nc.vector.tensor_add(v_col[:], v_tmp[:], bsrc[b - 1][:])

                # ---- topo qkv on raw v (PE, parallel with stats) ----
                nc.vector.tensor_copy(v_bf[:], v_col[:])
                nc.tensor.matmul(qk0_ps, v_bf[:], tqkv[:, 6144:6144 + 2 * S],
                                 start=True, stop=True)
                nc.tensor.matmul(A_ps[:, 2:3],
                                 tqkv[:, 6144 + 2 * S:6144 + 3 * S],
                                 v_bf[:], start=True, stop=True)

                # ---- stats + Newton-1 rsqrt ----
                nc.tensor.matmul(sv_ps, ones_col[:], v_col[:], start=True, stop=True)
                nc.tensor.matmul(svv_ps, v_col[:], v_col[:], start=True, stop=True)
                ts(sc[:, 0:1], sv_ps, 1.0 / S, mul_op)
                ts(sc[:, 1:2], svv_ps, 1.0 / S, mul_op)
                nc.vector.scalar_tensor_tensor(sc[:, 3:4], sc[:, 0:1], sc[:, 0:1],
                                               sc[:, 1:2], mul_op, sub_op)
                ts(sc[:, 4:5], sc[:, 3:4], -1.0, mul_op, EPS, add_op)      # vpe
                ts(sc[:, 5:6], sc[:, 3:4], -0.5, mul_op, 0.5 * EPS, add_op)  # vh
                ts(sci[:, 8:9], sci[:, 4:5], 1, shr_op)
                nc.vector.tensor_sub(sci[:, 6:7], magic[:, 0:1], sci[:, 8:9])
                nc.vector.scalar_tensor_tensor(sc[:, 8:9], sc[:, 6:7], sc[:, 5:6],
                                               sc[:, 6:7], mul_op, mul_op)
                ts(sc[:, 8:9], sc[:, 8:9], -1.0, mul_op, 1.5, add_op)
                nc.vector.tensor_mul(sc[:, 6:7], sc[:, 6:7], sc[:, 8:9])   # rstd
                nc.vector.tensor_mul(sc[:, 7:8], sc[:, 6:7], sc[:, 0:1])   # mu*rstd
                nc.tensor.matmul(bc_ps, ones_row[:], sc[:, 6:8], start=True, stop=True)
                nc.scalar.activation(bc_sb[:], bc_ps, Copy)
                rstd_c = bc_sb[:, 0:1]
                murstd_c = bc_sb[:, 1:2]

                # ---- u = rstd*gamma*(v-mu) + beta  (ACT) ----
                grstd = work.tile([S, 1], fp32, tag="grstd")
                gmr = work.tile([S, 1], fp32, tag="gmr")
                boff = work.tile([S, 1], fp32, tag="boff")
                ts(grstd[:], gam, rstd_c, mul_op)
                ts(gmr[:], gam, murstd_c, mul_op)
                nc.vector.tensor_sub(boff[:], bet, gmr[:])
                nc.scalar.activation(u_col[:], v_col[:], Ident,
                                     bias=boff[:, 0:1], scale=grstd[:, 0:1])

                # ---- topo attention: q,k corrected in row space (the
                # scalars live at partition 0 in sc, no broadcast needed) ----
                tc_row = trow[0:1, b * 512:b * 512 + 2 * S]
                bp_row = trow[0:1, b * 512 + 2 * S:(b + 1) * 512]
                nc.vector.scalar_tensor_tensor(cm2_row[:], tc_row, sc[:, 7:8],
                                               bp_row, mul_op, sub_op)
                nc.vector.scalar_tensor_tensor(qk_row[:], qk0_ps, sc[:, 6:7],
                                               cm2_row[:], mul_op, sub_op)
                nc.tensor.matmul(tsc_ps, qk_row[0:1, S:2 * S],
                                 qk_row[0:1, 0:S], start=True, stop=True)
                nc.vector.scalar_tensor_tensor(cmv[:], pre[:, 162 + b:163 + b],
                                               murstd_c[:, 0:1],
                                               pre[:, 170 + b:171 + b],
                                               mul_op, sub_op)
                nc.vector.scalar_tensor_tensor(qkvt_v[:], A_ps[:, 2:3],
                                               rstd_c[:, 0:1], cmv[:],
                                               mul_op, sub_op)
                nc.scalar.activation(Et_sb[:], tsc_ps, Exp, scale=RS)
                nc.scalar.activation(pvr_t[:, 0:1], qkvt_v[:], Copy)
                nc.tensor.matmul(pvt_ps, Et_sb[:], pvr_t[:], start=True, stop=True)
                rd1 = work.tile([S, 1], fp32, tag="rd1")
                nc.vector.reciprocal(rd1[:], pvt_ps[:, 1:2])
                nc.vector.scalar_tensor_tensor(up_col[:], pvt_ps[:, 0:1],
                                               rd1[:, 0:1], u_col[:],
                                               mul_op, add_op)
                nc.scalar.activation(up_bf[:], up_col[:], Copy)
                ts(Ub_bf[:], ones_mat_bf[:], up_col[:, 0:1], mul_op)

                # ---- neuron k,v columns (PE; stationary-load not the cost) ----
                for tl in range(TL):
                    base = 2048 + tl * 256
                    nc.tensor.matmul(kv_ps[:, 2 * tl:2 * tl + 1],
                                     tqkv[:, base:base + S],
                                     up_bf[:], start=True, stop=True)
                    nc.tensor.matmul(kv_ps[:, 2 * tl + 1:2 * tl + 2],
                                     tqkv[:, base + S:base + 2 * S],
                                     up_bf[:], start=True, stop=True)
                nc.vector.tensor_add(kvn[:], kv_ps, kvbias)
                k2 = kvn[:].rearrange("p (t k) -> p t k", k=2)
                ts(krsA[:], k2[:, 0:8, 0], RS, mul_op)
                ts(krsB[:], k2[:, 8:16, 0], RS, mul_op)
                p2 = pvr_n[:].rearrange("p (t k) -> p t k", k=2)
                nc.vector.tensor_mul(p2[:, :, 0], k2[:, :, 1], mt)
                nc.scalar.activation(p2[:, :, 1], mt, Copy)

                # ---- Q broadcast (+ q bias) in PSUM, bf16; fused-ACT half
                # (tls 8-15, scoresB) first so its exps start earliest ----
                for half, dst in ((1, scoresB), (0, scoresA)):
                    for hb in range(2):
                        ds = slice(hb * 512, (hb + 1) * 512)
                        cs = slice(half * 1024 + hb * 512,
                                   half * 1024 + (hb + 1) * 512)
                        nc.tensor.matmul(dst[:, ds], Ub_bf[:], tqkv[:, cs],
                                         start=True, stop=False,
                                         skip_group_check=True)
                        nc.tensor.matmul(dst[:, ds], ones_row_bf[:],
                                         bqr[:, cs], start=False, stop=True,
                                         skip_group_check=True)

                # ---- k*rs scale + exp: fused-ACT for tls 8-15 (tiles B),
                # DVE-scale + two wide ACT exps for tls 0-7 (tiles A).
                # Disjoint tiles per half so the scheduler can't tie the
                # engines together with tile-granular edges. ----
                for j in range(8):
                    tl = 8 + j
                    nc.scalar.activation(E_B[:, j * S:(j + 1) * S],
                                         scoresB[:, j * S:(j + 1) * S],
                                         Exp, scale=krsB[:, j:j + 1])
                    nc.tensor.matmul(pvn_ps[:, 2 * tl:2 * tl + 2],
                                     E_B[:, j * S:(j + 1) * S],
                                     pvr_n[:, 2 * tl:2 * tl + 2],
                                     start=True, stop=True)
                for half in range(2):
                    for j in range(4):
                        tl = 4 * half + j
                        ts(sc_sb[:, tl * S:(tl + 1) * S],
                           scoresA[:, tl * S:(tl + 1) * S],
                           krsA[:, tl:tl + 1], mul_op)
                    gs = slice(half * 512, (half + 1) * 512)
                    nc.scalar.activation(E_A[:, gs], sc_sb[:, gs], Exp)
                    for j in range(4):
                        tl = 4 * half + j
                        nc.tensor.matmul(pvn_ps[:, 2 * tl:2 * tl + 2],
                                         E_A[:, tl * S:(tl + 1) * S],
                                         pvr_n[:, 2 * tl:2 * tl + 2],
                                         start=True, stop=True)

                pv2 = pvn_ps.rearrange("p (t k) -> p t k", k=2)
                nc.vector.reciprocal(rden[:], pv2[:, :, 1])
                nc.vector.tensor_mul(zp[:], pv2[:, :, 0], rden[:])

                # ---- aff = sum_i wmt*(zp + u') + wbias  (fp32) ----
                nc.vector.tensor_mul(afr[:, 0:TL], wmt, zp[:])
                ts(afr[:, TL:2 * TL], wmt, up_col[:, 0:1], mul_op)
                nc.tensor.matmul(af_ps, afr[:, 0:TL], ones_col[:],
                                 start=True, stop=False, skip_group_check=True)
                nc.tensor.matmul(af_ps, afr[:, TL:2 * TL], ones_col[:],
                                 start=False, stop=False, skip_group_check=True)
                nc.tensor.matmul(af_ps, wbr[:, b * TL:(b + 1) * TL], one_one[:],
                                 start=False, stop=True, skip_group_check=True)

                if b == L - 1:
                    nc.vector.tensor_copy(aff_sb[:], af_ps)
                    nc.sync.dma_start(out_d, aff_sb[:])
                else:
                    nc.vector.tensor_copy(aff_sb[:], af_ps)
                    nc.tensor.matmul(v128_ps, sel[:], aff_sb[:],
                                     start=True, stop=True)
                    # adaptive gelu (tanh approx), sel-placed column
                    g0 = pre[:, 2 + b:3 + b]
                    g1h = pre[:, 10 + b:11 + b]
                    xg = work.tile([S, 1], fp32, tag="xg")
                    s2t = work.tile([S, 1], fp32, tag="s2t")
                    t1 = work.tile([S, 1], fp32, tag="t1")
                    ts(xg[:], v128_ps, g0[:, 0:1], mul_op)
                    nc.vector.tensor_mul(s2t[:], xg[:], xg[:])
                    ts(t1[:], s2t[:], GA, mul_op, 1.0, add_op)
                    nc.vector.tensor_mul(t1[:], t1[:], xg[:])
                    nc.scalar.activation(t1[:], t1[:], Tanh, scale=GC)
                    nc.vector.scalar_tensor_tensor(t1[:], t1[:], 1.0, xg[:],
                                                   add_op, mul_op)
                    # WAW anchor for the next batch's reduce: without it the
                    # scheduler hoists the (sem-blocked) reduce to the head
                    # of the in-order DVE queue and wedges the whole engine.
                    # Reads t1 (not bsrc) to stay off the trigger's WAR path.
                    ts(v_tmp[0:1, 0:1], ones_row[0:1, 0:1],
                       t1[0:1, 0:1], mul_op)
                    ts(bsrc[b][:], t1[:], g1h[:, 0:1], mul_op)
                    nc.gpsimd.trigger_dma(count=None,
                                          signals_writable=[bsrc[b][:]])

    nc.compile()
    return nc


def _host_prep(x, W, mask, attn_t, attn_n, norm_params, ada):
    import ml_dtypes
    f32 = np.float32
    bf16 = ml_dtypes.bfloat16
    x, W, mask, attn_t, attn_n, norm_params, ada = (
        np.ascontiguousarray(np.asarray(a, f32))
        for a in (x, W, mask, attn_t, attn_n, norm_params, ada))
    gamma = norm_params[:, 0, :]
    beta = norm_params[:, 1, :]

    topo_w = attn_t[:, :, :, :S]
    topo_b = attn_t[:, :, :, S]
    topo_wg = topo_w * gamma[:, None, None, :]
    topo_wt_flat = np.ascontiguousarray(
        topo_wg.transpose(0, 3, 1, 2)).reshape(L, S, 3 * S)
    topo_c = topo_wg.sum(axis=3)
    topo_bp = np.einsum('lmis,ls->lmi', topo_w, beta) + topo_b

    wmat = W[:, :, :S] * mask
    wbias = W[:, :, S]

    pre = np.zeros((S, 178), f32)
    pre[:, 0] = x
    pre[:, 2:10] = ada[:, :, 0].T
    pre[:, 10:18] = (0.5 * ada[:, :, 1]).astype(f32).T
    pre[:, 146:154] = gamma.T
    pre[:, 154:162] = beta.T
    pre[:, 162:170] = topo_c[:, 2, :].T
    pre[:, 170:178] = topo_bp[:, 2, :].T

    ident = np.eye(S, dtype=f32)
    magic = np.array([[MAGIC, 0]], np.int32)
    thr = np.full((1, 8), 14, np.int32)
    
    in_maps = []
    for c in range(N_CORES):
        sl = slice(c * TL, (c + 1) * TL)
        an = attn_n[:, sl]
        anw = an[:, :, :, :, :S]                              # (L,TL,3,i,p)
        anb = an[:, :, :, :, S]                               # (L,TL,3,i)
        qpart = np.ascontiguousarray(
            anw[:, :, 0].transpose(0, 3, 1, 2)).reshape(L, S, TL * S)
        kvpart = np.ascontiguousarray(
            anw[:, :, 1:3].transpose(0, 4, 1, 2, 3)).reshape(L, S, TL * 2 * S)
        kv = np.stack([anb[:, :, 1, :], anb[:, :, 2, :]], axis=2)  # (L,TL,2,i)
        kvb = kv.transpose(0, 3, 1, 2).reshape(L, S, 2 * TL)
        mtt = mask[:, sl].transpose(0, 2, 1)
        tqkv = np.concatenate([qpart, kvpart, topo_wt_flat,
                               kvb, mtt], axis=2).astype(bf16)
        premap = pre.copy()
        premap[:, 18:146] = wmat[:, sl].transpose(0, 2, 1).transpose(
            1, 0, 2).reshape(S, L * TL)
        bqr = np.ascontiguousarray(
            anb[:, :, 0, :].reshape(L, TL * S)).astype(bf16)
        tcqk = topo_c[:, 0:2, :].reshape(L, 2 * S)
        bpqk = topo_bp[:, 0:2, :].reshape(L, 2 * S)
        trow = np.ascontiguousarray(np.concatenate(
            [tcqk, bpqk], axis=1).reshape(1, L * 4 * S)).astype(bf16)
        selm = np.zeros((TL, S), f32)
        for j in range(TL):
            selm[j, c * TL + j] = 1.0
        wbr = np.ascontiguousarray(wbias[:, sl].reshape(1, L * TL))
        in_maps.append(dict(tqkv=tqkv, trow=trow,
                            bqr=bqr, pre=premap, sel=selm, wbr=wbr,
                            thr=thr, magic=magic))
    return in_maps


def kernel(x, W, mask, attn_t, attn_n, attn_mask_n, norm_params, ada,
           span_ids, tb_ids):
    global _cached
    _patch_topology()
    from concourse import bass_utils
    if _cached is None:
        _cached = _build()
    nc = _cached
    in_maps = _host_prep(x, W, mask, attn_t, attn_n, norm_params, ada)
    res = bass_utils.run_bass_kernel_spmd(nc, in_maps, core_ids=list(range(N_CORES)))
    out = np.concatenate([res.results[c]["out"].reshape(TL) for c in range(N_CORES)])
    return out.astype(np.float32)


# revision 8
# speedup vs baseline: 1.1046x; 1.0009x over previous
"""v3 Trainium2 Bass kernel.

Scheme (per core c of 8, per topo batch b of 8):
  gather v (remote-DMA all-gather, NOT gpsimd collective) -> layernorm stats
  (Newton-1 rsqrt) -> topo self-attention on the span -> per-neuron
  self-attention for this core's TL=16 neurons (bf16 matmuls; k*rs scale
  built as a PE matmul krep = krsT @ dsel and applied as one DVE multiply
  per 512-col bank, then one ACT exp per bank) -> masked affine (fp32)
  -> adaptive gelu on the sel-placed [128,1] column -> remote_dma_broadcast
  to all 8 cores' SBUF (slot k -> tpb my^k), wait on per-batch remote sem.

v3 changes vs v2 (327.7us):
  - All heavy matmuls bf16 (1 cy/row vs 4 for fp32); affine stays fp32.
  - gpsimd AllGather (15us/call in the cost model) replaced by 8
    single-dest remote_dma_broadcast preps + one trigger per batch
    (~1-2us); per-batch remote semaphores, register-valued wait threshold
    (schedule-time sim cannot constant-fold it).
  - k*rs fused via krep matmul instead of 16 per-tl tensor_scalars.
  - Copies/casts moved to ACT; stats chain shortened; single Newton iter.
"""
import sys
import numpy as np

sys.path.insert(0, "/opt/trn_rl_repo")

I, L, T, S = 128, 8, 128, 128
N_CORES = 8
TL = T // N_CORES
EPS = 1e-5
RS = float(1.0 / np.sqrt(np.float32(S)))
GC = 0.7978845608028654
GA = 0.044715
MAGIC = 0x5F3759DF

_cached = None


def _patch_topology():
    """No /dev/neuron* client-side: give the sim the static TRN2 NC map it
    needs to route remote DMA (the NEFF itself uses relative XOR routing)."""
    from concourse import libnrt
    base = (0, 1, 2, 3, 6, 7, 4, 5)

    def get_trn2_nc_mapping():
        return {(d, k): base[k] for d in range(16) for k in range(8)}

    def nc_to_real_nc(device_index, nc_index):
        return base[nc_index]

    def pnc_id_to_device_and_real_nc_index(core_id):
        return core_id // 8, base[core_id % 8]

    def get_device_id_to_routing_id_mapping():
        return {d: d for d in range(16)}

    libnrt.get_trn2_nc_mapping = get_trn2_nc_mapping
    libnrt.nc_to_real_nc = nc_to_real_nc
    libnrt.pnc_id_to_device_and_real_nc_index = pnc_id_to_device_and_real_nc_index
    libnrt.get_device_id_to_routing_id_mapping = get_device_id_to_routing_id_mapping
    for modname in ("concourse.bass_interp", "concourse.dge_state"):
        m = sys.modules.get(modname)
        if m is None:
            continue
        for fn in (nc_to_real_nc, pnc_id_to_device_and_real_nc_index,
                   get_device_id_to_routing_id_mapping):
            if hasattr(m, fn.__name__):
                setattr(m, fn.__name__, fn)


def _build():
    _patch_topology()
    from concourse import bacc, tile, mybir

    fp32 = mybir.dt.float32
    bf16 = mybir.dt.bfloat16
    int32 = mybir.dt.int32
    Exp = mybir.ActivationFunctionType.Exp
    Tanh = mybir.ActivationFunctionType.Tanh
    Copy = mybir.ActivationFunctionType.Copy
    Ident = mybir.ActivationFunctionType.Identity
    mul_op = mybir.AluOpType.mult
    add_op = mybir.AluOpType.add
    sub_op = mybir.AluOpType.subtract
    shr_op = mybir.AluOpType.arith_shift_right
    AxX = mybir.AxisListType.X

    nc = bacc.Bacc("TRN2", target_bir_lowering=False, debug=False,
                   enable_asserts=True, num_devices=N_CORES)

    tqkv_d = nc.dram_tensor("tqkv", [L, S, 6576], bf16,
                            kind="ExternalInput").ap()  # Q | k,v | topo | kvbias,mt
    bqr_d = nc.dram_tensor("bqr", [L, TL * S], bf16, kind="ExternalInput").ap()
    trow_d = nc.dram_tensor("trow", [1, L * 4 * S], bf16, kind="ExternalInput").ap()
    pre_d = nc.dram_tensor("pre", [S, 178], fp32, kind="ExternalInput").ap()
    sel_d = nc.dram_tensor("sel", [TL, S], fp32, kind="ExternalInput").ap()
    wbr_d = nc.dram_tensor("wbr", [1, L * TL], fp32, kind="ExternalInput").ap()
    thr_d = nc.dram_tensor("thr", [1, 8], int32, kind="ExternalInput").ap()
    magic_d = nc.dram_tensor("magic", [1, 2], int32, kind="ExternalInput").ap()
    out_d = nc.dram_tensor("out", [TL, 1], fp32, kind="ExternalOutput").ap()

    rsems = [nc.alloc_semaphore(f"rsem{b}") for b in range(L - 1)]
    lsem = nc.alloc_semaphore("lsem")
    bsem = nc.alloc_semaphore("bsem")
    gsem = nc.alloc_semaphore("gsem")

    with tile.TileContext(nc) as tc:
        with tc.tile_pool(name="wpool", bufs=3) as wpool, \
             tc.tile_pool(name="spool", bufs=3) as spool, \
             tc.tile_pool(name="fixed", bufs=1) as fixed, \
             tc.tile_pool(name="work", bufs=1) as work, \
             tc.tile_pool(name="ps_big", bufs=1, space="PSUM") as ps_big, \
             tc.tile_pool(name="ps_sm", bufs=1, space="PSUM") as ps_sm, \
             tc.tile_pool(name="ps_tp", bufs=1, space="PSUM") as ps_tp:

            pre = fixed.tile([S, 178], fp32)
            nc.sync.dma_start(pre[:], pre_d)
            magic = fixed.tile([1, 2], int32)
            nc.scalar.dma_start(magic[:], magic_d)
            trow = fixed.tile([1, L * 4 * S], bf16)
            nc.scalar.dma_start(trow[:], trow_d)
            thr = fixed.tile([1, 8], int32)
            sel = fixed.tile([TL, S], fp32)
            wbr = fixed.tile([1, L * TL], fp32)
            ones_col = fixed.tile([S, 1], fp32)
            nc.vector.memset(ones_col[:], 1.0)
            ones_row = fixed.tile([1, S], fp32)
            nc.vector.memset(ones_row[:], 1.0)
            ones_row_bf = fixed.tile([1, S], bf16)
            nc.vector.memset(ones_row_bf[:], 1.0)
            ones_mat_bf = fixed.tile([S, S], bf16)
            nc.vector.memset(ones_mat_bf[:], 1.0)
            one_one = fixed.tile([1, 1], fp32)
            nc.vector.memset(one_one[:], 1.0)
            pvr_t = fixed.tile([S, 2], bf16)
            nc.vector.memset(pvr_t[:], 1.0)

            bsrc = [fixed.tile([S, 1], fp32, name=f"bsrc{b}") for b in range(L - 1)]
            v8s = [fixed.tile([S, 7], fp32, name=f"v8_{b}") for b in range(L - 1)]

            v_col = work.tile([S, 1], fp32)
            u_col = work.tile([S, 1], fp32)
            up_col = work.tile([S, 1], fp32)
            up_bf = work.tile([S, 1], bf16)
            v_bf = work.tile([S, 1], bf16)
            Ub_bf = work.tile([S, S], bf16)
            sc = work.tile([1, 12], fp32)
            sci = sc[:].bitcast(int32)
            bc_sb = work.tile([S, 2], fp32)
            qkvt_c = work.tile([S, 3], fp32)
            qk_row = work.tile([1, 2 * S], fp32)
            cm2_row = work.tile([1, 2 * S], fp32)
            cmv = work.tile([S, 1], fp32)
            qkvt_v = work.tile([S, 1], fp32)
            Et_sb = work.tile([S, S], bf16)
            v_tmp = work.tile([S, 1], fp32)
            kvn = work.tile([S, 2 * TL], fp32)
            krsA = work.tile([S, TL // 2], fp32)
            krsB = work.tile([S, TL // 2], fp32)
            pvr_n = work.tile([S, 2 * TL], bf16)
            sc_sb = work.tile([S, TL * S // 2], bf16)
            E_A = work.tile([S, TL * S // 2], bf16)
            E_B = work.tile([S, TL * S // 2], bf16)
            rden = work.tile([S, TL], fp32)
            zp = work.tile([S, TL], fp32)
            afr = work.tile([S, 2 * TL], fp32)
            aff_sb = work.tile([TL, 1], fp32)

            scoresA = ps_big.tile([S, 1024], fp32)       # 2 banks (tls 0-7)
            scoresB = ps_big.tile([S, 1024], fp32)       # 2 banks (tls 8-15)
            smps = ps_sm.tile([S, 512], fp32)            # 1 bank
            kv_ps = smps[:, 0:32]
            pvn_ps = smps[:, 32:64]
            af_ps = smps[0:TL, 64:65]
            v128_ps = smps[:, 66:67]
            sv_ps = smps[0:1, 68:69]
            svv_ps = smps[0:1, 69:70]
            bc_ps = smps[:, 70:72]
            A_ps = smps[:, 72:75]
            pvt_ps = smps[:, 76:78]
            tpps = ps_tp.tile([S, 512], fp32)            # 1 bank (topo stage)
            qk0_ps = tpps[0:1, 128:384]
            tsc_ps = tpps[:, 0:128]

            rthr_cm = nc.vector.register("rthr")
            rthr = rthr_cm.__enter__()
            nc.vector.reg_load(rthr, thr[0:1, 0:1])

            # Tracked WAW edge: reg_save writes a byte of v_col, so every
            # later v_col writer (incl. the sem-waiting reduce) orders after
            # the reg_load (register deps inside wait conditions are not
            # tracked by tile).
            nc.vector.reg_save(v_tmp[0:1, 0:1].bitcast(int32), rthr)

            def ts(out, in0, s1, op0, s2=None, op1=None, eng=None):
                e = eng or nc.vector
                if s2 is None:
                    e.tensor_scalar(out, in0, s1, None, op0)
                else:
                    e.tensor_scalar(out, in0, s1, s2, op0, op1)

            for b in range(L):
                # ---- weight prefetch (tqkv split in 4 so the gather trigger
                # never queues behind a >1us DMA) ----
                tqkv = wpool.tile([S, 6576], bf16, tag="tqkv")
                bqr = spool.tile([1, TL * S], bf16, tag="bqr")
                for q in ((2, 0) if b == 0 else range(3)):
                    nc.sync.dma_start(tqkv[:, q * 2192:(q + 1) * 2192],
                                      tqkv_d[b][:, q * 2192:(q + 1) * 2192])
                if b == 0:
                    nc.scalar.dma_start(tqkv[:, 2192:2 * 2192],
                                        tqkv_d[b][:, 2192:2 * 2192])
                if b > 0:
                    nc.sync.dma_start(bqr[:], bqr_d[b])
                if b == 0:
                    nc.sync.dma_start(thr[:], thr_d)
                    nc.sync.dma_start(bqr[:], bqr_d[b])
                    # Tracked WAW edge: reg_save writes a byte of v_tmp, so
                    # every later v_tmp writer (incl. the sem-waiting reduce)
                    # orders after the reg_load (register deps inside wait
                    # conditions are not tracked by tile).
                    nc.vector.reg_load(rthr, thr[0:1, 0:1])
                    nc.vector.reg_save(v_tmp[0:1, 0:1].bitcast(int32), rthr)
                    # Tracked WAW edge: reg_save writes a byte of v_tmp, so
                    # every later v_tmp writer (incl. the sem-waiting reduce)
                    # orders after the reg_load (register deps inside wait
                    # conditions are not tracked by tile).
                    nc.vector.reg_load(rthr, thr[0:1, 0:1])
                    nc.vector.reg_save(v_tmp[0:1, 0:1].bitcast(int32), rthr)
                if b == 0:
                    # deferred: keeps the early ACT queue clear for batch 0;
                    # these land ~12us, first use ~15us (batch-0 tail)
                    nc.scalar.dma_start(sel[:], sel_d)
                    nc.scalar.dma_start(wbr[:], wbr_d)
                kvbias = tqkv[:, 6528:6560]
                mt = tqkv[:, 6560:6576]
                wmt = pre[:, 18 + 16 * b:18 + 16 * (b + 1)]
                gam = pre[:, 146 + b:147 + b]
                bet = pre[:, 154 + b:155 + b]

                # ---- desc-gen for THIS batch's end-of-batch broadcast (the
                # trigger at the end of this batch fires these 8 preps) ----
                if b < L - 1:
                    if b >= 2:
                        # SWDGE ring holds ~14 preps. Dummy write to bsrc[b]
                        # reading bsrc[b-2] (a declared output of trigger
                        # b-2): the preps' no-sync src edge then orders them
                        # after trigger b-2 on the in-order Pool queue, so
                        # ring entries are reclaimed before desc-gen.
                        ts(bsrc[b][0:1, 0:1], ones_row[0:1, 0:1],
                           bsrc[b - 2][0:1, 0:1], mul_op)
                    for k in range(1, N_CORES):
                        rd = [None] * 8
                        rd[k] = (0, k)
                        nc.gpsimd.remote_dma_broadcast(
                            v8s[b][:, k - 1:k], bsrc[b][:],
                            rsems[b], lsem, rdests=rd)

                # ---- acquire v ----
                if b == 0:
                    nc.vector.tensor_copy(v_col[:], pre[:, 0:1])
                else:
                    red = nc.vector.tensor_reduce(v_tmp[:], v8s[b - 1][:],
                                                  AxX, add_op)
                    red.wait_op(rsems[b - 1], rthr, "sem-ge")
                    nc.vector.tensor_add(v_col[:], v_tmp[:], bsrc[b - 1][:])

                # ---- topo qkv on raw v (PE, parallel with stats) ----
                nc.vector.tensor_copy(v_bf[:], v_col[:])
                nc.tensor.matmul(qk0_ps, v_bf[:], tqkv[:, 6144:6144 + 2 * S],
                                 start=True, stop=True)
                nc.tensor.matmul(A_ps[:, 2:3],
                                 tqkv[:, 6144 + 2 * S:6144 + 3 * S],
                                 v_bf[:], start=True, stop=True)

                # ---- stats + Newton-1 rsqrt ----
                nc.tensor.matmul(sv_ps, ones_col[:], v_col[:], start=True, stop=True)
                nc.tensor.matmul(svv_ps, v_col[:], v_col[:], start=True, stop=True)
                ts(sc[:, 0:1], sv_ps, 1.0 / S, mul_op)
                ts(sc[:, 1:2], svv_ps, 1.0 / S, mul_op)
                nc.vector.scalar_tensor_tensor(sc[:, 3:4], sc[:, 0:1], sc[:, 0:1],
                                               sc[:, 1:2], mul_op, sub_op)
                ts(sc[:, 4:5], sc[:, 3:4], -1.0, mul_op, EPS, add_op)      # vpe
                ts(sc[:, 5:6], sc[:, 3:4], -0.5, mul_op, 0.5 * EPS, add_op)  # vh
                ts(sci[:, 8:9], sci[:, 4:5], 1, shr_op)
                nc.vector.tensor_sub(sci[:, 6:7], magic[:, 0:1], sci[:, 8:9])
                nc.vector.scalar_tensor_tensor(sc[:, 8:9], sc[:, 6:7], sc[:, 5:6],
                                               sc[:, 6:7], mul_op, mul_op)
                ts(sc[:, 8:9], sc[:, 8:9], -1.0, mul_op, 1.5, add_op)
                nc.vector.tensor_mul(sc[:, 6:7], sc[:, 6:7], sc[:, 8:9])   # rstd
                nc.vector.tensor_mul(sc[:, 7:8], sc[:, 6:7], sc[:, 0:1])   # mu*rstd
                nc.tensor.matmul(bc_ps, ones_row[:], sc[:, 6:8], start=True, stop=True)
                nc.vector.tensor_copy(bc_sb[:], bc_ps)
                rstd_c = bc_sb[:, 0:1]
                murstd_c = bc_sb[:, 1:2]

                # ---- u = rstd*gamma*(v-mu) + beta  (ACT) ----
                grstd = work.tile([S, 1], fp32, tag="grstd")
                gmr = work.tile([S, 1], fp32, tag="gmr")
                boff = work.tile([S, 1], fp32, tag="boff")
                ts(grstd[:], gam, rstd_c, mul_op)
                ts(gmr[:], gam, murstd_c, mul_op)
                nc.vector.tensor_sub(boff[:], bet, gmr[:])
                nc.vector.scalar_tensor_tensor(u_col[:], v_col[:],
                                               grstd[:, 0:1], boff[:, 0:1],
                                               mul_op, add_op)

                # ---- topo attention: q,k corrected in row space (the
                # scalars live at partition 0 in sc, no broadcast needed) ----
                tc_row = trow[0:1, b * 512:b * 512 + 2 * S]
                bp_row = trow[0:1, b * 512 + 2 * S:(b + 1) * 512]
                nc.vector.scalar_tensor_tensor(cm2_row[:], tc_row, sc[:, 7:8],
                                               bp_row, mul_op, sub_op)
                nc.vector.scalar_tensor_tensor(qk_row[:], qk0_ps, sc[:, 6:7],
                                               cm2_row[:], mul_op, sub_op)
                nc.tensor.matmul(tsc_ps, qk_row[0:1, S:2 * S],
                                 qk_row[0:1, 0:S], start=True, stop=True)
                nc.vector.scalar_tensor_tensor(cmv[:], pre[:, 162 + b:163 + b],
                                               murstd_c[:, 0:1],
                                               pre[:, 170 + b:171 + b],
                                               mul_op, sub_op)
                nc.vector.scalar_tensor_tensor(qkvt_v[:], A_ps[:, 2:3],
                                               rstd_c[:, 0:1], cmv[:],
                                               mul_op, sub_op)
                nc.scalar.activation(Et_sb[:], tsc_ps, Exp, scale=RS)
                nc.vector.tensor_copy(pvr_t[:, 0:1], qkvt_v[:])
                nc.tensor.matmul(pvt_ps, Et_sb[:], pvr_t[:], start=True, stop=True)
                rd1 = work.tile([S, 1], fp32, tag="rd1")
                nc.vector.reciprocal(rd1[:], pvt_ps[:, 1:2])
                nc.vector.scalar_tensor_tensor(up_col[:], pvt_ps[:, 0:1],
                                               rd1[:, 0:1], u_col[:],
                                               mul_op, add_op)
                nc.vector.tensor_copy(up_bf[:], up_col[:])
                ts(Ub_bf[:], ones_mat_bf[:], up_col[:, 0:1], mul_op)

                # ---- neuron k,v columns (PE; stationary-load not the cost) ----
                for tl in range(TL):
                    base = 2048 + tl * 256
                    nc.tensor.matmul(kv_ps[:, 2 * tl:2 * tl + 1],
                                     tqkv[:, base:base + S],
                                     up_bf[:], start=True, stop=True)
                    nc.tensor.matmul(kv_ps[:, 2 * tl + 1:2 * tl + 2],
                                     tqkv[:, base + S:base + 2 * S],
                                     up_bf[:], start=True, stop=True)
                nc.vector.tensor_add(kvn[:], kv_ps, kvbias)
                k2 = kvn[:].rearrange("p (t k) -> p t k", k=2)
                ts(krsA[:], k2[:, 0:8, 0], RS, mul_op)
                ts(krsB[:], k2[:, 8:16, 0], RS, mul_op)
                p2 = pvr_n[:].rearrange("p (t k) -> p t k", k=2)
                nc.vector.tensor_mul(p2[:, :, 0], k2[:, :, 1], mt)
                nc.scalar.activation(p2[:, :, 1], mt, Copy)

                # ---- Q broadcast (+ q bias) in PSUM, bf16; fused-ACT half
                # (tls 8-15, scoresB) first so its exps start earliest ----
                for half, dst in ((1, scoresB), (0, scoresA)):
                    for hb in range(2):
                        ds = slice(hb * 512, (hb + 1) * 512)
                        cs = slice(half * 1024 + hb * 512,
                                   half * 1024 + (hb + 1) * 512)
                        nc.tensor.matmul(dst[:, ds], Ub_bf[:], tqkv[:, cs],
                                         start=True, stop=False,
                                         skip_group_check=True)
                        nc.tensor.matmul(dst[:, ds], ones_row_bf[:],
                                         bqr[:, cs], start=False, stop=True,
                                         skip_group_check=True)

                # ---- k*rs scale + exp: fused-ACT for tls 8-15 (tiles B),
                # DVE-scale + two wide ACT exps for tls 0-7 (tiles A).
                # Disjoint tiles per half so the scheduler can't tie the
                # engines together with tile-granular edges. ----
                for j in range(8):
                    tl = 8 + j
                    nc.scalar.activation(E_B[:, j * S:(j + 1) * S],
                                         scoresB[:, j * S:(j + 1) * S],
                                         Exp, scale=krsB[:, j:j + 1])
                    nc.tensor.matmul(pvn_ps[:, 2 * tl:2 * tl + 2],
                                     E_B[:, j * S:(j + 1) * S],
                                     pvr_n[:, 2 * tl:2 * tl + 2],
                                     start=True, stop=True)
                for half in range(2):
                    for j in range(4):
                        tl = 4 * half + j
                        ts(sc_sb[:, tl * S:(tl + 1) * S],
                           scoresA[:, tl * S:(tl + 1) * S],
                           krsA[:, tl:tl + 1], mul_op)
                    gs = slice(half * 512, (half + 1) * 512)
                    nc.scalar.activation(E_A[:, gs], sc_sb[:, gs], Exp)
                    for j in range(4):
                        tl = 4 * half + j
                        nc.tensor.matmul(pvn_ps[:, 2 * tl:2 * tl + 2],
                                         E_A[:, tl * S:(tl + 1) * S],
                                         pvr_n[:, 2 * tl:2 * tl + 2],
                                         start=True, stop=True)

                pv2 = pvn_ps.rearrange("p (t k) -> p t k", k=2)
                nc.vector.reciprocal(rden[:], pv2[:, :, 1])
                nc.vector.tensor_mul(zp[:], pv2[:, :, 0], rden[:])

                # ---- aff = sum_i wmt*(zp + u') + wbias  (fp32) ----
                nc.vector.tensor_mul(afr[:, 0:TL], wmt, zp[:])
                ts(afr[:, TL:2 * TL], wmt, up_col[:, 0:1], mul_op)
                nc.tensor.matmul(af_ps, afr[:, 0:TL], ones_col[:],
                                 start=True, stop=False, skip_group_check=True)
                nc.tensor.matmul(af_ps, afr[:, TL:2 * TL], ones_col[:],
                                 start=False, stop=False, skip_group_check=True)
                nc.tensor.matmul(af_ps, wbr[:, b * TL:(b + 1) * TL], one_one[:],
                                 start=False, stop=True, skip_group_check=True)

                if b == L - 1:
                    nc.vector.tensor_copy(aff_sb[:], af_ps)
                    nc.sync.dma_start(out_d, aff_sb[:])
                else:
                    nc.vector.tensor_copy(aff_sb[:], af_ps)
                    nc.tensor.matmul(v128_ps, sel[:], aff_sb[:],
                                     start=True, stop=True)
                    # adaptive gelu (tanh approx), sel-placed column
                    g0 = pre[:, 2 + b:3 + b]
                    g1h = pre[:, 10 + b:11 + b]
                    xg = work.tile([S, 1], fp32, tag="xg")
                    s2t = work.tile([S, 1], fp32, tag="s2t")
                    t1 = work.tile([S, 1], fp32, tag="t1")
                    ts(xg[:], v128_ps, g0[:, 0:1], mul_op)
                    nc.vector.tensor_mul(s2t[:], xg[:], xg[:])
                    ts(t1[:], s2t[:], GA, mul_op, 1.0, add_op)
                    nc.vector.tensor_mul(t1[:], t1[:], xg[:])
                    nc.scalar.activation(t1[:], t1[:], Tanh, scale=GC)
                    nc.vector.scalar_tensor_tensor(t1[:], t1[:], 1.0, xg[:],
                                                   add_op, mul_op)
                    # WAW anchor for the next batch's reduce: without it the
                    # scheduler hoists the (sem-blocked) reduce to the head
                    # of the in-order DVE queue and wedges the whole engine.
                    # Reads t1 (not bsrc) to stay off the trigger's WAR path.
                    ts(v_tmp[0:1, 0:1], ones_row[0:1, 0:1],
                       t1[0:1, 0:1], mul_op)
                    ts(bsrc[b][:], t1[:], g1h[:, 0:1], mul_op)
                    nc.gpsimd.trigger_dma(count=None,
                                          signals_writable=[bsrc[b][:]])

    nc.compile()
    return nc


def _host_prep(x, W, mask, attn_t, attn_n, norm_params, ada):
    import ml_dtypes
    f32 = np.float32
    bf16 = ml_dtypes.bfloat16
    x, W, mask, attn_t, attn_n, norm_params, ada = (
        np.ascontiguousarray(np.asarray(a, f32))
        for a in (x, W, mask, attn_t, attn_n, norm_params, ada))
    gamma = norm_params[:, 0, :]
    beta = norm_params[:, 1, :]

    topo_w = attn_t[:, :, :, :S]
    topo_b = attn_t[:, :, :, S]
    topo_wg = topo_w * gamma[:, None, None, :]
    topo_wt_flat = np.ascontiguousarray(
        topo_wg.transpose(0, 3, 1, 2)).reshape(L, S, 3 * S)
    topo_c = topo_wg.sum(axis=3)
    topo_bp = np.einsum('lmis,ls->lmi', topo_w, beta) + topo_b

    wmat = W[:, :, :S] * mask
    wbias = W[:, :, S]

    pre = np.zeros((S, 178), f32)
    pre[:, 0] = x
    pre[:, 2:10] = ada[:, :, 0].T
    pre[:, 10:18] = (0.5 * ada[:, :, 1]).astype(f32).T
    pre[:, 146:154] = gamma.T
    pre[:, 154:162] = beta.T
    pre[:, 162:170] = topo_c[:, 2, :].T
    pre[:, 170:178] = topo_bp[:, 2, :].T

    ident = np.eye(S, dtype=f32)
    magic = np.array([[MAGIC, 0]], np.int32)
    thr = np.full((1, 8), 14, np.int32)
    
    in_maps = []
    for c in range(N_CORES):
        sl = slice(c * TL, (c + 1) * TL)
        an = attn_n[:, sl]
        anw = an[:, :, :, :, :S]                              # (L,TL,3,i,p)
        anb = an[:, :, :, :, S]                               # (L,TL,3,i)
        qpart = np.ascontiguousarray(
            anw[:, :, 0].transpose(0, 3, 1, 2)).reshape(L, S, TL * S)
        kvpart = np.ascontiguousarray(
            anw[:, :, 1:3].transpose(0, 4, 1, 2, 3)).reshape(L, S, TL * 2 * S)
        kv = np.stack([anb[:, :, 1, :], anb[:, :, 2, :]], axis=2)  # (L,TL,2,i)
        kvb = kv.transpose(0, 3, 1, 2).reshape(L, S, 2 * TL)
        mtt = mask[:, sl].transpose(0, 2, 1)
        tqkv = np.concatenate([qpart, kvpart, topo_wt_flat,
                               kvb, mtt], axis=2).astype(bf16)
        premap = pre.copy()
        premap[:, 18:146] = wmat[:, sl].transpose(0, 2, 1).transpose(
            1, 0, 2).reshape(S, L * TL)
        bqr = np.ascontiguousarray(
            anb[:, :, 0, :].reshape(L, TL * S)).astype(bf16)
        tcqk = topo_c[:, 0:2, :].reshape(L, 2 * S)
        bpqk = topo_bp[:, 0:2, :].reshape(L, 2 * S)
        trow = np.ascontiguousarray(np.concatenate(
            [tcqk, bpqk], axis=1).reshape(1, L * 4 * S)).astype(bf16)
        selm = np.zeros((TL, S), f32)
        for j in range(TL):
            selm[j, c * TL + j] = 1.0
        wbr = np.ascontiguousarray(wbias[:, sl].reshape(1, L * TL))
        in_maps.append(dict(tqkv=tqkv, trow=trow,
                            bqr=bqr, pre=premap, sel=selm, wbr=wbr,
                            thr=thr, magic=magic))
    return in_maps


def kernel(x, W, mask, attn_t, attn_n, attn_mask_n, norm_params, ada,
           span_ids, tb_ids):
    global _cached
    _patch_topology()
    from concourse import bass_utils
    if _cached is None:
        _cached = _build()
    nc = _cached
    in_maps = _host_prep(x, W, mask, attn_t, attn_n, norm_params, ada)
    res = bass_utils.run_bass_kernel_spmd(nc, in_maps, core_ids=list(range(N_CORES)))
    out = np.concatenate([res.results[c]["out"].reshape(TL) for c in range(N_CORES)])
    return out.astype(np.float32)
